# revision 25
# baseline (speedup 1.0000x reference)
"""CapsuleNet Trainium2 kernel.

Data-parallel over batch: 64 items -> 8 cores x 8 items. Weights replicated.

Math (per item), matching the reference:
  e   = emb[x] * mask                      [L=512, E=512]
  h   = relu(conv1d(e.T, k=9, pad=4) + b1) [C=32, L=512]
  p   = conv1d(h, k=9, pad=4, stride=2)+b2 [UC=256, S=256]
  p   = squash(p over C-blocks of 32)
  routing (R=3) with b (logits) independent of S:
    c[u,k] = softmax_k(b);  s[k] = sum_u c[u,k] * (W[u,k].T @ p_u)
    v[k] = squash_c(s[k]);  agree[u,k] = <W[u,k], p_u.T @ v[k]>;  b += agree
  out = mean_s(v)                          [K=9, C=32]

Key layout: everything channel-on-partitions, position-on-free:
  eT   [128e x 4, 520]  (4-col zero pad both sides for conv shifts)
  h    [32, 520]        (padded)
  p    [128 x 2, 256]   rows = (u_local*32 + c'), chunk ch = u//4
  s,v  [128, 256] x 3   rows = (k_local*32 + c), kgroups (k0-3, k4-7, k8)
u_hat is never materialized: s is computed straight from p with a
c-weighted W (lhsT), and agree via Gram matrices G = pT.T @ vT.
"""

import numpy as np

import concourse.bass as bass
import concourse.tile as tile
from concourse import bacc, mybir
from concourse.bass_utils import run_bass_kernel_spmd

F32 = mybir.dt.float32
I32 = mybir.dt.int32
AF = mybir.ActivationFunctionType
ALU = mybir.AluOpType
AX = mybir.AxisListType

V, E, L = 50000, 512, 512
B, U, C, K, R = 64, 8, 32, 9, 3
S = 256
NCORES = 8
BL = B // NCORES  # items per core
F32R = mybir.dt.float32r

# conv1 tap assignment to the 4 PE column groups (M=32 each)
TGROUPS = [[0, 4, 8], [1, 5], [2, 6], [3, 7]]
# routing k-groups: (cols in the 288-wide (k,c) axis, partition count)
KGS = [(0, 128), (128, 128), (256, 32)]


def _emit(tc, nc, aps, bl):
    from contextlib import ExitStack

    es = ExitStack()
    x_ap = aps["x"]
    mask_ap = aps["mask"]
    emb_ap = aps["emb"]
    out_ap = aps["out"]

    F32R = mybir.dt.float32r

    def MM(out, lhsT, rhs, **kw):
        return nc.tensor.matmul(
            out=out, lhsT=lhsT.bitcast(F32R), rhs=rhs.bitcast(F32R), **kw
        )

    def TP(out, in_, identity, **kw):
        return nc.tensor.transpose(
            out=out.bitcast(F32R), in_=in_.bitcast(F32R),
            identity=identity.bitcast(F32R), **kw
        )

    cp = es.enter_context(tc.tile_pool(name="consts", bufs=1))
    w1 = cp.tile([128, 1152], F32R)
    nc.sync.dma_start(out=w1[:], in_=aps["w1"])
    w2 = cp.tile([32, 2304], F32R)
    nc.sync.dma_start(out=w2[:], in_=aps["w2"])
    wf = cp.tile([128, 576], F32R)
    nc.sync.dma_start(out=wf[:], in_=aps["wf"])
    w9 = cp.tile([128, 576], F32R)
    nc.sync.dma_start(out=w9[:], in_=aps["w9"])
    b1 = cp.tile([32, 1], F32)
    nc.sync.dma_start(out=b1[:], in_=aps["b1"])
    b2 = cp.tile([128, 2], F32)
    nc.sync.dma_start(out=b2[:], in_=aps["b2"])
    ident = cp.tile([128, 128], F32R)
    nc.sync.dma_start(out=ident[:], in_=aps["ident"])
    ind_sq8 = cp.tile([128, 16], F32R)
    nc.sync.dma_start(out=ind_sq8[:], in_=aps["ind_sq8"])
    indT8 = cp.tile([8, 256], F32R)
    nc.sync.dma_start(out=indT8[:], in_=aps["indT8"])
    kind = cp.tile([128, 36], F32R)
    nc.sync.dma_start(out=kind[:], in_=aps["kind"])
    kindT = cp.tile([12, 288], F32R)
    nc.sync.dma_start(out=kindT[:], in_=aps["kindT"])
    uind = cp.tile([128, 4], F32R)
    nc.sync.dma_start(out=uind[:], in_=aps["uind"])
    u4T = cp.tile([4, 128], F32R)
    nc.sync.dma_start(out=u4T[:], in_=aps["u4T"])
    fb = cp.tile([128, 2], F32)  # col0 = 1e-8 (eps), col1 = 1.0
    nc.sync.dma_start(out=fb[:], in_=aps["fb"])

    gp = es.enter_context(tc.tile_pool(name="gather", bufs=2))
    wp = es.enter_context(tc.tile_pool(name="work", bufs=2))
    sp = es.enter_context(tc.tile_pool(name="small", bufs=2))
    pq = es.enter_context(tc.tile_pool(name="persist", bufs=1))
    pp = es.enter_context(tc.tile_pool(name="psum", bufs=2, space="PSUM"))

    PS = [None] * bl   # squashed p tiles [2][128, 256] per item
    PT = [None] * bl   # p transposed [2][128 s, 256 (u,c')]
    BT = [None] * bl   # routing logits [2][4, 9]
    WCS = [None] * bl  # c-weighted W [2][128, 288]
    VS = [None] * bl   # current v tiles [3]
    SSB = [None] * bl

    def stage_a(it):
        # ---- embedding gather (+mask), [l, e] layout ----
        em_tiles = []
        for lc in range(4):
            idx = gp.tile([128, 1], I32, tag="idx")
            nc.sync.dma_start(
                out=idx[:], in_=x_ap[it, lc * 128 : (lc + 1) * 128].unsqueeze(1)
            )
            msk = gp.tile([128, 1], F32, tag="msk")
            nc.sync.dma_start(
                out=msk[:],
                in_=mask_ap[it, lc * 128 : (lc + 1) * 128].unsqueeze(1),
            )
            eraw = gp.tile([128, 512], F32, tag="eraw")
            nc.gpsimd.indirect_dma_start(
                out=eraw[:],
                out_offset=None,
                in_=emb_ap,
                in_offset=bass.IndirectOffsetOnAxis(ap=idx[:, 0:1], axis=0),
            )
            em = gp.tile([128, 512], F32, tag=f"em{lc}")
            nc.scalar.activation(out=em[:].bitcast(F32R), in_=eraw[:], func=AF.Copy, scale=msk[:, 0:1])
            em_tiles.append(em)

        # ---- transpose to eT [e, l] with 4-col pad each side ----
        eT = []
        for ec in range(4):
            t = gp.tile([128, 520], F32, tag=f"eT{ec}")
            nc.vector.memset(t[:, 0:4], 0.0)
            nc.vector.memset(t[:, 516:520], 0.0)
            eT.append(t)
        for lc in range(4):
            for ec in range(4):
                ps = pp.tile([128, 128], F32, tag="tp")
                TP(
                    out=ps[:], in_=em_tiles[lc][:, ec * 128 : (ec + 1) * 128],
                    identity=ident[:],
                )
                nc.scalar.copy(
                    out=eT[ec][:, 4 + lc * 128 : 4 + (lc + 1) * 128].bitcast(F32R), in_=ps[:]
                )

        # ---- conv1: fp32 shifted-rhs matmuls, 4 PE column groups ----
        psy = pp.tile([128, 512], F32, tag="cva")
        for j, taps in enumerate(TGROUPS):
            n = len(taps) * 4
            cnt = 0
            for t in taps:
                for ec in range(4):
                    nc.tensor.matmul(
                        out=psy[32 * j : 32 * (j + 1), :],
                        lhsT=w1[:, t * 128 + ec * 32 : t * 128 + (ec + 1) * 32].bitcast(F32),
                        rhs=eT[ec][:, t : t + 512],
                        start=(cnt == 0),
                        stop=(cnt == n - 1),
                        tile_position=(0, 32 * j),
                    )
                    cnt += 1
        # collapse 4 col-groups + bias + relu -> h [32, 520] padded
        c1 = wp.tile([32, 512], F32, tag="c1")
        nc.scalar.copy(out=c1[:], in_=psy[32:64, :])
        c3 = wp.tile([32, 512], F32, tag="c3")
        nc.scalar.copy(out=c3[:], in_=psy[96:128, :])
        a0 = wp.tile([32, 512], F32, tag="a0")
        nc.vector.tensor_add(out=a0[:], in0=psy[0:32, :], in1=c1[:])
        a1 = wp.tile([32, 512], F32, tag="a1")
        nc.vector.tensor_add(out=a1[:], in0=psy[64:96, :], in1=c3[:])
        ya = wp.tile([32, 512], F32, tag="ya")
        nc.vector.tensor_add(out=ya[:], in0=a0[:], in1=a1[:])
        hp = wp.tile([32, 520], F32, tag="hp")
        nc.vector.memset(hp[:, 0:4], 0.0)
        nc.vector.memset(hp[:, 516:520], 0.0)
        nc.scalar.activation(
            out=hp[:, 4:516].bitcast(F32R), in_=ya[:], func=AF.Relu, bias=b1[:, 0:1]
        )

        # ---- primary caps conv (stride 2): p [(u,c') = 128 x 2, 256] ----
        psp = pp.tile([128, 512], F32, tag="pra", bufs=1)
        for h in range(2):
            for t in range(9):
                rhs = hp[:, t : t + 512].rearrange("p (s two) -> p s two", two=2)[:, :, 0]
                MM(
                    out=psp[:, h * 256 : (h + 1) * 256],
                    lhsT=w2[:, t * 256 + h * 128 : t * 256 + (h + 1) * 128],
                    rhs=rhs,
                    start=(t == 0),
                    stop=(t == 8),
                )
        psb, p2 = [], []
        for h in range(2):
            sb = wp.tile([128, 256], F32, tag=f"psb{h}")
            nc.scalar.activation(
                out=sb[:], in_=psp[:, h * 256 : (h + 1) * 256], func=AF.Identity,
                bias=b2[:, h : h + 1],
            )
            psb.append(sb)
            q = wp.tile([128, 256], F32, tag=f"p2{h}")
            nc.scalar.square(out=q[:].bitcast(F32R), in_=sb[:])
            p2.append(q)
        psq = pp.tile([8, 256], F32, tag="sm", bufs=1)
        MM(out=psq[:], lhsT=ind_sq8[:, 0:8], rhs=p2[0][:], start=True, stop=False)
        MM(out=psq[:], lhsT=ind_sq8[:, 8:16], rhs=p2[1][:], start=False, stop=True)
        t1 = sp.tile([8, 256], F32, tag="t1")
        nc.scalar.activation(out=t1[:], in_=psq[:], func=AF.Sqrt, bias=fb[0:8, 0:1])
        t2 = sp.tile([8, 256], F32, tag="t2")
        nc.scalar.activation(out=t2[:], in_=psq[:], func=AF.Identity, bias=fb[0:8, 1:2])
        t3 = sp.tile([8, 256], F32, tag="t3")
        nc.vector.tensor_mul(out=t3[:], in0=t1[:], in1=t2[:])
        t4 = sp.tile([8, 256], F32, tag="t4")
        nc.vector.reciprocal(out=t4[:], in_=t3[:])
        f8 = sp.tile([8, 256], F32, tag="f8")
        nc.vector.tensor_mul(out=f8[:].bitcast(F32R), in0=psq[:], in1=t4[:])
        ps_t, pT = [], []
        for h in range(2):
            pfb = pp.tile([128, 256], F32, tag="rt")
            MM(
                out=pfb[:], lhsT=indT8[:, h * 128 : (h + 1) * 128], rhs=f8[:],
                start=True, stop=True,
            )
            pst = pq.tile([128, 256], F32, tag=f"ps{h}_{it}", name=f"ps{h}_{it}")
            nc.vector.tensor_mul(out=pst[:].bitcast(F32R), in0=psb[h][:], in1=pfb[:])
            ps_t.append(pst)
        for sc in range(2):
            t = pq.tile([128, 256], F32, tag=f"pT{sc}_{it}", name=f"pT{sc}_{it}")
            pT.append(t)
        for h in range(2):
            for sc in range(2):
                tp = pp.tile([128, 128], F32, tag="tp")
                TP(
                    out=tp[:], in_=ps_t[h][:, sc * 128 : (sc + 1) * 128],
                    identity=ident[:],
                )
                nc.scalar.copy(out=pT[sc][:, h * 128 : (h + 1) * 128].bitcast(F32R), in_=tp[:])
        PS[it] = ps_t
        PT[it] = pT
        BT[it] = [
            pq.tile([4, 9], F32, tag=f"bt{ch}_{it}", name=f"bt{ch}_{it}")
            for ch in range(2)
        ]

    def softmax_wcs(it):
        b_t = BT[it]
        wcs = []
        for ch in range(2):
            negm = sp.tile([4, 1], F32, tag="negm")
            nc.vector.reduce_max(out=negm[:], in_=b_t[ch][:], axis=AX.X, negate=True)
            ex = sp.tile([4, 9], F32, tag="ex")
            nc.scalar.activation(out=ex[:], in_=b_t[ch][:], func=AF.Exp, bias=negm[:, 0:1])
            sm = sp.tile([4, 1], F32, tag="sm9")
            nc.vector.reduce_sum(out=sm[:], in_=ex[:], axis=AX.X)
            rs = sp.tile([4, 1], F32, tag="rs")
            nc.vector.reciprocal(out=rs[:], in_=sm[:])
            cc = sp.tile([4, 9], F32, tag="cc")
            nc.vector.tensor_scalar_mul(out=cc[:], in0=ex[:], scalar1=rs[:, 0:1])
            csm = sp.tile([4, 288], F32, tag="csm")
            nc.vector.tensor_copy(
                out=csm[:].rearrange("p (k c) -> p k c", c=32).bitcast(F32R),
                in_=cc[:].unsqueeze(2).to_broadcast([4, 9, 32]),
            )
            sbc = pp.tile([128, 288], F32, tag="rt")
            MM(out=sbc[:], lhsT=u4T[:], rhs=csm[:], start=True, stop=True)
            wc = wp.tile([128, 288], F32, tag=f"wcs{ch}")
            nc.vector.tensor_mul(out=wc[:].bitcast(F32R), in0=wf[:, ch * 288 : (ch + 1) * 288].bitcast(F32), in1=sbc[:])
            wcs.append(wc)
        WCS[it] = wcs

    def s_and_v(it, r):
        ps_t = PS[it]

        def lhs_s(ch, c0, c1):
            if r == 0:
                return w9[:, ch * 288 + c0 : ch * 288 + c1]
            return WCS[it][ch][:, c0:c1]

        s_sb, s2 = [], []
        for kg, (c0, m) in enumerate(KGS):
            sps = pp.tile([m, 256], F32, tag="rt")
            for ch in range(2):
                MM(
                    out=sps[:], lhsT=lhs_s(ch, c0, c0 + m), rhs=ps_t[ch][:],
                    start=(ch == 0), stop=(ch == 1),
                )
            ssb = wp.tile([m, 256], F32, tag=f"ssb{kg}", bufs=3)
            nc.scalar.copy(out=ssb[:], in_=sps[:])
            s_sb.append(ssb)
            q = wp.tile([m, 256], F32, tag=f"s2{kg}")
            nc.scalar.square(out=q[:].bitcast(F32R), in_=sps[:])
            s2.append(q)
        sqk = pp.tile([12, 256], F32, tag="sm", bufs=1)
        for kg, (c0, m) in enumerate(KGS):
            MM(
                out=sqk[:], lhsT=kind[0:m, kg * 12 : (kg + 1) * 12], rhs=s2[kg][:],
                start=(kg == 0), stop=(kg == 2),
            )
        u1 = sp.tile([12, 256], F32, tag="u1")
        nc.scalar.activation(out=u1[:], in_=sqk[:], func=AF.Sqrt, bias=fb[0:12, 0:1])
        u2 = sp.tile([12, 256], F32, tag="u2")
        nc.scalar.activation(out=u2[:], in_=sqk[:], func=AF.Identity, bias=fb[0:12, 1:2])
        u3 = sp.tile([12, 256], F32, tag="u3")
        nc.vector.tensor_mul(out=u3[:], in0=u1[:], in1=u2[:])
        u4 = sp.tile([12, 256], F32, tag="u4")
        nc.vector.reciprocal(out=u4[:], in_=u3[:])
        fk = sp.tile([12, 256], F32, tag="fk")
        nc.vector.tensor_mul(out=fk[:].bitcast(F32R), in0=sqk[:], in1=u4[:])
        if r == R - 1:
            fks = sp.tile([12, 256], F32, tag="fks")
            nc.scalar.mul(out=fks[:].bitcast(F32R), in_=fk[:].bitcast(F32), mul=1.0 / S)
            fk = fks
        v_sb = []
        for kg, (c0, m) in enumerate(KGS):
            vfb = pp.tile([m, 256], F32, tag="rt")
            MM(
                out=vfb[:], lhsT=kindT[:, c0 : c0 + m], rhs=fk[:],
                start=True, stop=True,
            )
            vkg = wp.tile([m, 256], F32, tag=f"v{kg}", bufs=3)
            nc.vector.tensor_mul(out=vkg[:].bitcast(F32R), in0=s_sb[kg][:], in1=vfb[:])
            v_sb.append(vkg)
        VS[it] = v_sb
        SSB[it] = s_sb

    def agree_update(it, r):
        pT = PT[it]
        v_sb = VS[it]
        b_t = BT[it]
        vT = [
            wp.tile([128, 288], F32, tag=f"vT{sc}", name=f"vT{sc}_{it}_{r}")
            for sc in range(2)
        ]
        for kg, (c0, m) in enumerate(KGS):
            for sc in range(2):
                tp = pp.tile([128, m], F32, tag="tp")
                TP(
                    out=tp[:], in_=v_sb[kg][:, sc * 128 : (sc + 1) * 128],
                    identity=ident[0:m, 0:m],
                )
                nc.scalar.copy(out=vT[sc][:, c0 : c0 + m].bitcast(F32R), in_=tp[:])
        for ch in range(2):
            gps = pp.tile([128, 288], F32, tag="rt")
            for sc in range(2):
                MM(
                    out=gps[:], lhsT=pT[sc][:, ch * 128 : (ch + 1) * 128],
                    rhs=vT[sc][:], start=(sc == 0), stop=(sc == 1),
                )
            ga = wp.tile([128, 288], F32, tag=f"ga{ch}")
            nc.vector.tensor_mul(out=ga[:].bitcast(F32R), in0=wf[:, ch * 288 : (ch + 1) * 288].bitcast(F32), in1=gps[:])
            aps_ = pp.tile([4, 288], F32, tag="sm", bufs=1)
            MM(out=aps_[:], lhsT=uind[:], rhs=ga[:], start=True, stop=True)
            agr = sp.tile([4, 9], F32, tag=f"agr{ch}")
            nc.vector.reduce_sum(
                out=agr[:], in_=aps_[:].rearrange("p (k c) -> p k c", c=32),
                axis=AX.X,
            )
            if r == 0:
                nc.vector.tensor_copy(out=b_t[ch][:], in_=agr[:])
            else:
                nc.vector.tensor_add(out=b_t[ch][:], in0=b_t[ch][:], in1=agr[:])

    def emit_out(it):
        for kg, (c0, m) in enumerate(KGS):
            vm = sp.tile([m, 1], F32, tag=f"vm{kg}")
            nc.vector.reduce_sum(out=vm[:], in_=VS[it][kg][:], axis=AX.X)
            nc.sync.dma_start(
                out=out_ap[it, c0 : c0 + m].unsqueeze(1),
                in_=vm[:, 0:1],
            )

    for it in range(bl):
        stage_a(it)
    for r in range(R):
        for it in range(bl):
            if r > 0:
                softmax_wcs(it)
            s_and_v(it, r)
            if r < R - 1:
                agree_update(it, r)
            else:
                emit_out(it)
    es.close()


def _pack_consts(inputs):
    conv1_w = np.ascontiguousarray(np.asarray(inputs["conv1_w"], np.float32))
    conv1_b = np.asarray(inputs["conv1_b"], np.float32)
    prim_w = np.ascontiguousarray(np.asarray(inputs["prim_w"], np.float32))
    prim_b = np.asarray(inputs["prim_b"], np.float32)
    W = np.asarray(inputs["W"], np.float32)

    w1 = np.zeros((128, 1152), np.float32)
    for t in range(9):
        for ec in range(4):
            w1[:, t * 128 + ec * 32 : t * 128 + (ec + 1) * 32] = conv1_w[
                :, ec * 128 : (ec + 1) * 128, t
            ].T
    w2 = np.zeros((32, 2304), np.float32)
    for t in range(9):
        w2[:, t * 256 : (t + 1) * 256] = prim_w[:, :, t].T
    wfr = W[0].transpose(0, 2, 1, 3).reshape(U, C, K * C)  # [u, c', (k c)]
    wf = np.zeros((128, 576), np.float32)
    for ch in range(2):
        wf[:, ch * 288 : (ch + 1) * 288] = wfr[ch * 4 : (ch + 1) * 4].reshape(128, 288)
    w9 = wf / 9.0
    b1 = conv1_b.reshape(32, 1).copy()
    b2 = prim_b.reshape(2, 128).T.copy()
    ident = np.eye(128, dtype=np.float32)

    ind_sq8 = np.zeros((128, 16), np.float32)
    for p in range(128):
        ind_sq8[p, p // 32] = 1.0
        ind_sq8[p, 12 + p // 32] = 1.0
    indT8 = np.zeros((8, 256), np.float32)
    for p in range(128):
        indT8[p // 32, p] = 1.0
        indT8[4 + p // 32, 128 + p] = 1.0
    kind = np.zeros((128, 36), np.float32)
    for kg in range(3):
        m = 128 if kg < 2 else 32
        for p in range(m):
            kind[p, kg * 12 + kg * 4 + p // 32] = 1.0
    kindT = np.zeros((12, 288), np.float32)
    for kg in range(3):
        m = 128 if kg < 2 else 32
        for p in range(m):
            kindT[kg * 4 + p // 32, kg * 128 + p] = 1.0
    uind = np.zeros((128, 4), np.float32)
    for p in range(128):
        uind[p, p // 32] = 1.0
    u4T = np.zeros((4, 128), np.float32)
    for p in range(128):
        u4T[p // 32, p] = 1.0

    ind36 = np.zeros((36, 256), np.float32)
    for ch in range(2):
        for p in range(128):
            ind36[ch * 32 + p // 32, ch * 128 + p] = 1.0

    fbc = np.zeros((128, 2), np.float32)
    fbc[:, 0] = 1e-8
    fbc[:, 1] = 1.0

    return {
        "w1": w1, "w2": w2, "wf": wf, "w9": w9, "b1": b1, "b2": b2,
        "ident": ident, "ind_sq8": ind_sq8, "indT8": indT8, "kind": kind,
        "kindT": kindT, "uind": uind, "u4T": u4T, "fb": fbc, "ind36": ind36,
    }


_NC_CACHE = {}


def build_nc(bl=BL):
    if bl in _NC_CACHE:
        return _NC_CACHE[bl]
    nc = bacc.Bacc(
        "TRN2", target_bir_lowering=False, debug=False, num_devices=NCORES
    )
    shapes = {
        "x": ([bl, L], I32), "mask": ([bl, L], F32), "emb": ([V, E], F32),
        "w1": ([128, 1152], F32R), "w2": ([32, 2304], F32R), "wf": ([128, 576], F32R),
        "w9": ([128, 576], F32R), "b1": ([32, 1], F32), "b2": ([128, 2], F32),
        "ident": ([128, 128], F32R), "ind_sq8": ([128, 16], F32R),
        "indT8": ([8, 256], F32R), "kind": ([128, 36], F32R),
        "kindT": ([12, 288], F32R), "uind": ([128, 4], F32R), "u4T": ([4, 128], F32R),
        "fb": ([128, 2], F32), "ind36": ([36, 256], F32R),
    }
    aps = {
        name: nc.dram_tensor(name, shp, dt, kind="ExternalInput").ap()
        for name, (shp, dt) in shapes.items()
    }
    aps["out"] = nc.dram_tensor("out", [bl, K * C], F32, kind="ExternalOutput").ap()
    with tile.TileContext(nc) as tc:
        _emit(tc, nc, aps, bl)
    nc.compile()
    _NC_CACHE[bl] = nc
    return nc


def make_in_maps(inputs, bl=BL, ncores=NCORES):
    consts = _pack_consts(inputs)
    x = np.ascontiguousarray(np.asarray(inputs["x"], np.int32).reshape(ncores, bl, L))
    mask = np.ascontiguousarray(
        np.asarray(inputs["attention_mask"], np.float32).reshape(ncores, bl, L)
    )
    emb = np.ascontiguousarray(np.asarray(inputs["emb"], np.float32))
    return [
        {"x": x[i], "mask": mask[i], "emb": emb, **consts} for i in range(ncores)
    ]


def kernel(x, attention_mask, emb, conv1_w, conv1_b, prim_w, prim_b, W):
    inputs = {
        "x": x, "attention_mask": attention_mask, "emb": emb,
        "conv1_w": conv1_w, "conv1_b": conv1_b,
        "prim_w": prim_w, "prim_b": prim_b, "W": W,
    }
    nc = build_nc(BL)
    in_maps = make_in_maps(inputs)
    res = run_bass_kernel_spmd(nc, in_maps, core_ids=list(range(NCORES)))
    out = np.concatenate(
        [res.results[i]["out"].reshape(BL, K, C) for i in range(NCORES)], axis=0
    )
    return out.astype(np.float32)


# revision 32
# speedup vs baseline: 1.2116x; 1.2116x over previous
"""CapsuleNet Trainium2 kernel.

Data-parallel over batch: 64 items -> 8 cores x 8 items. Weights replicated.

Math (per item), matching the reference:
  e   = emb[x] * mask                      [L=512, E=512]
  h   = relu(conv1d(e.T, k=9, pad=4) + b1) [C=32, L=512]
  p   = conv1d(h, k=9, pad=4, stride=2)+b2 [UC=256, S=256]
  p   = squash(p over C-blocks of 32)
  routing (R=3) with b (logits) independent of S:
    c[u,k] = softmax_k(b);  s[k] = sum_u c[u,k] * (W[u,k].T @ p_u)
    v[k] = squash_c(s[k]);  agree[u,k] = <W[u,k], p_u.T @ v[k]>;  b += agree
  out = mean_s(v)                          [K=9, C=32]

Key layout: everything channel-on-partitions, position-on-free:
  eT   [128e x 4, 520]  (4-col zero pad both sides for conv shifts)
  h    [32, 520]        (padded)
  p    [128 x 2, 256]   rows = (u_local*32 + c'), chunk ch = u//4
  s,v  [128, 256] x 3   rows = (k_local*32 + c), kgroups (k0-3, k4-7, k8)
u_hat is never materialized: s is computed straight from p with a
c-weighted W (lhsT), and agree via Gram matrices G = pT.T @ vT.
"""

import numpy as np

import concourse.bass as bass
import concourse.tile as tile
from concourse import bacc, mybir
from concourse.bass_utils import run_bass_kernel_spmd

F32 = mybir.dt.float32
I32 = mybir.dt.int32
AF = mybir.ActivationFunctionType
ALU = mybir.AluOpType
AX = mybir.AxisListType

V, E, L = 50000, 512, 512
B, U, C, K, R = 64, 8, 32, 9, 3
S = 256
NCORES = 8
BL = B // NCORES  # items per core
F32R = mybir.dt.float32r

# conv1 tap assignment to the 4 PE column groups (M=32 each)
TGROUPS = [[0, 4, 8], [1, 5], [2, 6], [3, 7]]
# routing k-groups: (cols in the 288-wide (k,c) axis, partition count)
KGS = [(0, 128), (128, 128), (256, 32)]


def _emit(tc, nc, aps, bl):
    from contextlib import ExitStack

    es = ExitStack()
    x_ap = aps["x"]
    mask_ap = aps["mask"]
    emb_ap = aps["emb"]
    out_ap = aps["out"]

    F32R = mybir.dt.float32r

    def MM(out, lhsT, rhs, **kw):
        return nc.tensor.matmul(
            out=out, lhsT=lhsT.bitcast(F32R), rhs=rhs.bitcast(F32R), **kw
        )

    def TP(out, in_, identity, **kw):
        return nc.tensor.transpose(
            out=out.bitcast(F32R), in_=in_.bitcast(F32R),
            identity=identity.bitcast(F32R), **kw
        )

    cp = es.enter_context(tc.tile_pool(name="consts", bufs=1))
    w1 = cp.tile([128, 1152], F32R)
    nc.sync.dma_start(out=w1[:], in_=aps["w1"])
    w2 = cp.tile([32, 2304], F32R)
    nc.sync.dma_start(out=w2[:], in_=aps["w2"])
    wf = cp.tile([128, 576], F32R)
    nc.sync.dma_start(out=wf[:], in_=aps["wf"])
    w9 = cp.tile([128, 576], F32R)
    nc.sync.dma_start(out=w9[:], in_=aps["w9"])
    b1 = cp.tile([32, 1], F32)
    nc.sync.dma_start(out=b1[:], in_=aps["b1"])
    b2 = cp.tile([128, 2], F32)
    nc.sync.dma_start(out=b2[:], in_=aps["b2"])
    ident = cp.tile([128, 128], F32R)
    nc.sync.dma_start(out=ident[:], in_=aps["ident"])
    ind_sq8 = cp.tile([128, 16], F32R)
    nc.sync.dma_start(out=ind_sq8[:], in_=aps["ind_sq8"])
    indT8 = cp.tile([8, 256], F32R)
    nc.sync.dma_start(out=indT8[:], in_=aps["indT8"])
    kind = cp.tile([128, 36], F32R)
    nc.sync.dma_start(out=kind[:], in_=aps["kind"])
    kindT = cp.tile([12, 288], F32R)
    nc.sync.dma_start(out=kindT[:], in_=aps["kindT"])
    uind = cp.tile([128, 4], F32R)
    nc.sync.dma_start(out=uind[:], in_=aps["uind"])
    u4T = cp.tile([4, 128], F32R)
    nc.sync.dma_start(out=u4T[:], in_=aps["u4T"])
    fb = cp.tile([128, 2], F32)  # col0 = 1e-8 (eps), col1 = 1.0
    nc.sync.dma_start(out=fb[:], in_=aps["fb"])

    gp = es.enter_context(tc.tile_pool(name="gather", bufs=2))
    wp = es.enter_context(tc.tile_pool(name="work", bufs=2))
    sp = es.enter_context(tc.tile_pool(name="small", bufs=2))
    pq = es.enter_context(tc.tile_pool(name="persist", bufs=1))
    pp = es.enter_context(tc.tile_pool(name="psum", bufs=2, space="PSUM"))

    HP = [None] * bl
    PSB = [None] * bl
    P2 = [None] * bl
    PSQ = [None] * bl
    F8 = [None] * bl
    PS = [None] * bl
    PT = [None] * bl
    BT = [None] * bl
    WCS = [None] * bl
    SSB = [None] * bl
    S2 = [None] * bl
    SQK = [None] * bl
    FK = [None] * bl
    VS = [None] * bl
    VT = [None] * bl

    # ---------- stage A1: gather, mask-diag transpose, conv1, collapse ----------
    def a1(it):
        eraws = []
        for lc in range(4):
            idx = gp.tile([128, 1], I32, tag="idx", bufs=2)
            nc.sync.dma_start(
                out=idx[:], in_=x_ap[it, lc * 128 : (lc + 1) * 128].unsqueeze(1)
            )
            msk = gp.tile([128, 1], F32, tag="msk", bufs=2)
            nc.sync.dma_start(
                out=msk[:],
                in_=mask_ap[it, lc * 128 : (lc + 1) * 128].unsqueeze(1),
            )
            eraw = gp.tile([128, 512], F32R, tag="eraw", bufs=4)
            nc.gpsimd.indirect_dma_start(
                out=eraw[:],
                out_offset=None,
                in_=emb_ap,
                in_offset=bass.IndirectOffsetOnAxis(ap=idx[:, 0:1], axis=0),
            )
            dmask = gp.tile([128, 128], F32, tag="dmask", bufs=4)
            nc.vector.tensor_scalar_mul(
                out=dmask[:].bitcast(F32R), in0=ident[:].bitcast(F32), scalar1=msk[:, 0:1]
            )
            eraws.append((eraw, dmask))
        eT = []
        for ec in range(4):
            t = gp.tile([128, 520], F32, tag=f"eT{ec}")
            nc.vector.memset(t[:, 0:4], 0.0)
            nc.vector.memset(t[:, 516:520], 0.0)
            eT.append(t)
        for lc in range(4):
            eraw, dmask = eraws[lc]
            for ec in range(4):
                ps = pp.tile([128, 128], F32, tag="tp")
                TP(
                    out=ps[:], in_=eraw[:, ec * 128 : (ec + 1) * 128],
                    identity=dmask[:],
                )
                nc.scalar.copy(
                    out=eT[ec][:, 4 + lc * 128 : 4 + (lc + 1) * 128].bitcast(F32R), in_=ps[:]
                )
        psy = pp.tile([128, 512], F32, tag="cva")
        for j, taps in enumerate(TGROUPS):
            n = len(taps) * 4
            cnt = 0
            for t in taps:
                for ec in range(4):
                    nc.tensor.matmul(
                        out=psy[32 * j : 32 * (j + 1), :],
                        lhsT=w1[:, t * 128 + ec * 32 : t * 128 + (ec + 1) * 32].bitcast(F32),
                        rhs=eT[ec][:, t : t + 512],
                        start=(cnt == 0),
                        stop=(cnt == n - 1),
                        tile_position=(0, 32 * j),
                    )
                    cnt += 1
        c1 = wp.tile([32, 512], F32, tag="c1", bufs=1)
        nc.scalar.copy(out=c1[:], in_=psy[32:64, :])
        c3 = wp.tile([32, 512], F32, tag="c3", bufs=1)
        nc.scalar.copy(out=c3[:], in_=psy[96:128, :])
        a0 = wp.tile([32, 512], F32, tag="a0")
        nc.vector.tensor_add(out=a0[:], in0=psy[0:32, :], in1=c1[:])
        a1_ = wp.tile([32, 512], F32, tag="a1")
        nc.vector.tensor_add(out=a1_[:], in0=psy[64:96, :], in1=c3[:])
        ya = wp.tile([32, 512], F32, tag="ya", bufs=1)
        nc.vector.tensor_add(out=ya[:], in0=a0[:], in1=a1_[:])
        hp = pq.tile([32, 520], F32, tag=f"hp{it}", name=f"hp{it}")
        nc.vector.memset(hp[:, 0:4], 0.0)
        nc.vector.memset(hp[:, 516:520], 0.0)
        nc.scalar.activation(
            out=hp[:, 4:516].bitcast(F32R), in_=ya[:], func=AF.Relu, bias=b1[:, 0:1]
        )
        HP[it] = hp

    # ---------- stage A2: primary conv + bias + square ----------
    def a2(it):
        hp = HP[it]
        psp = pp.tile([128, 512], F32, tag="pra")
        for h in range(2):
            for t in range(9):
                rhs = hp[:, t : t + 512].rearrange("p (s two) -> p s two", two=2)[:, :, 0]
                MM(
                    out=psp[:, h * 256 : (h + 1) * 256],
                    lhsT=w2[:, t * 256 + h * 128 : t * 256 + (h + 1) * 128],
                    rhs=rhs,
                    start=(t == 0),
                    stop=(t == 8),
                )
        psb, p2 = [], []
        for h in range(2):
            sb = wp.tile([128, 256], F32, tag=f"psb{h}", bufs=3)
            nc.scalar.activation(
                out=sb[:], in_=psp[:, h * 256 : (h + 1) * 256], func=AF.Identity,
                bias=b2[:, h : h + 1],
            )
            psb.append(sb)
            q = wp.tile([128, 256], F32, tag=f"p2{h}", bufs=3)
            nc.scalar.square(out=q[:].bitcast(F32R), in_=sb[:])
            p2.append(q)
        PSB[it] = psb
        P2[it] = p2
        psq = pp.tile([8, 256], F32, tag="rtsm")
        MM(out=psq[:], lhsT=ind_sq8[:, 0:8], rhs=p2[0][:], start=True, stop=False)
        MM(out=psq[:], lhsT=ind_sq8[:, 8:16], rhs=p2[1][:], start=False, stop=True)
        t1 = sp.tile([8, 256], F32, tag="t1")
        nc.scalar.activation(out=t1[:], in_=psq[:], func=AF.Sqrt, bias=fb[0:8, 0:1])
        t2 = sp.tile([8, 256], F32, tag="t2")
        nc.scalar.activation(out=t2[:], in_=psq[:], func=AF.Identity, bias=fb[0:8, 1:2])
        t3 = sp.tile([8, 256], F32, tag="t3")
        nc.vector.tensor_mul(out=t3[:], in0=t1[:], in1=t2[:])
        t4 = sp.tile([8, 256], F32, tag="t4")
        nc.vector.reciprocal(out=t4[:], in_=t3[:])
        f8 = sp.tile([8, 256], F32, tag="f8", bufs=3)
        nc.vector.tensor_mul(out=f8[:].bitcast(F32R), in0=psq[:], in1=t4[:])
        F8[it] = f8

    # ---------- stage A4: broadcast, squash-mult, transpose p ----------
    def a4(it):
        psb = PSB[it]
        ps_t, pT = [], []
        for h in range(2):
            pfb = pp.tile([128, 256], F32, tag="rtsm")
            MM(
                out=pfb[:], lhsT=indT8[:, h * 128 : (h + 1) * 128], rhs=F8[it][:],
                start=True, stop=True,
            )
            pst = pq.tile([128, 256], F32, tag=f"ps{h}_{it}", name=f"ps{h}_{it}")
            nc.vector.tensor_mul(out=pst[:].bitcast(F32R), in0=psb[h][:], in1=pfb[:])
            ps_t.append(pst)
        for sc in range(2):
            t = pq.tile([128, 256], F32, tag=f"pT{sc}_{it}", name=f"pT{sc}_{it}")
            pT.append(t)
        for h in range(2):
            for sc in range(2):
                tp = pp.tile([128, 128], F32, tag="tp")
                TP(
                    out=tp[:], in_=ps_t[h][:, sc * 128 : (sc + 1) * 128],
                    identity=ident[:],
                )
                nc.scalar.copy(out=pT[sc][:, h * 128 : (h + 1) * 128].bitcast(F32R), in_=tp[:])
        PS[it] = ps_t
        PT[it] = pT
        BT[it] = [
            pq.tile([4, 9], F32, tag=f"bt{ch}_{it}", name=f"bt{ch}_{it}")
            for ch in range(2)
        ]

    # ---------- routing phases ----------
    def softmax_wcs(it):
        b_t = BT[it]
        wcs = []
        for ch in range(2):
            negm = sp.tile([4, 1], F32, tag="negm")
            nc.vector.reduce_max(out=negm[:], in_=b_t[ch][:], axis=AX.X, negate=True)
            ex = sp.tile([4, 9], F32, tag="ex")
            nc.scalar.activation(out=ex[:], in_=b_t[ch][:], func=AF.Exp, bias=negm[:, 0:1])
            sm = sp.tile([4, 1], F32, tag="sm9")
            nc.vector.reduce_sum(out=sm[:], in_=ex[:], axis=AX.X)
            rs = sp.tile([4, 1], F32, tag="rs")
            nc.vector.reciprocal(out=rs[:], in_=sm[:])
            cc = sp.tile([4, 9], F32, tag="cc")
            nc.vector.tensor_scalar_mul(out=cc[:], in0=ex[:], scalar1=rs[:, 0:1])
            csm = sp.tile([4, 288], F32, tag="csm")
            nc.vector.tensor_copy(
                out=csm[:].rearrange("p (k c) -> p k c", c=32).bitcast(F32R),
                in_=cc[:].unsqueeze(2).to_broadcast([4, 9, 32]),
            )
            sbc = pp.tile([128, 288], F32, tag="rtsm")
            MM(out=sbc[:], lhsT=u4T[:], rhs=csm[:], start=True, stop=True)
            wc = wp.tile([128, 288], F32, tag=f"wcs{ch}", bufs=3)
            nc.vector.tensor_mul(out=wc[:].bitcast(F32R), in0=wf[:, ch * 288 : (ch + 1) * 288].bitcast(F32), in1=sbc[:])
            wcs.append(wc)
        WCS[it] = wcs

    def s_phase(it, r):
        ps_t = PS[it]

        def lhs_s(ch, c0, c1):
            if r == 0:
                return w9[:, ch * 288 + c0 : ch * 288 + c1]
            return WCS[it][ch][:, c0:c1]

        s_sb, s2 = [], []
        for kg, (c0, m) in enumerate(KGS):
            sps = pp.tile([m, 256], F32, tag="rtsm")
            for ch in range(2):
                MM(
                    out=sps[:], lhsT=lhs_s(ch, c0, c0 + m), rhs=ps_t[ch][:],
                    start=(ch == 0), stop=(ch == 1),
                )
            ssb = wp.tile([m, 256], F32, tag=f"ssb{kg}", bufs=3)
            nc.scalar.copy(out=ssb[:], in_=sps[:])
            s_sb.append(ssb)
            q = wp.tile([m, 256], F32, tag=f"s2{kg}", bufs=3)
            nc.scalar.square(out=q[:].bitcast(F32R), in_=sps[:])
            s2.append(q)
        SSB[it] = s_sb
        S2[it] = s2

    def sqk_phase(it, r):
        s2 = S2[it]
        sqk = pp.tile([12, 256], F32, tag="rtsm")
        for kg, (c0, m) in enumerate(KGS):
            MM(
                out=sqk[:], lhsT=kind[0:m, kg * 12 : (kg + 1) * 12], rhs=s2[kg][:],
                start=(kg == 0), stop=(kg == 2),
            )
        u1 = sp.tile([12, 256], F32, tag="u1")
        nc.scalar.activation(out=u1[:], in_=sqk[:], func=AF.Sqrt, bias=fb[0:12, 0:1])
        u2 = sp.tile([12, 256], F32, tag="u2")
        nc.scalar.activation(out=u2[:], in_=sqk[:], func=AF.Identity, bias=fb[0:12, 1:2])
        u3 = sp.tile([12, 256], F32, tag="u3")
        nc.vector.tensor_mul(out=u3[:], in0=u1[:], in1=u2[:])
        u4 = sp.tile([12, 256], F32, tag="u4")
        nc.vector.reciprocal(out=u4[:], in_=u3[:])
        fk = sp.tile([12, 256], F32, tag="fk", bufs=3)
        nc.vector.tensor_mul(out=fk[:].bitcast(F32R), in0=sqk[:], in1=u4[:])
        if r == R - 1:
            fks = sp.tile([12, 256], F32, tag="fks", bufs=3)
            nc.scalar.mul(out=fks[:].bitcast(F32R), in_=fk[:].bitcast(F32), mul=1.0 / S)
            fk = fks
        FK[it] = fk

    def v_phase(it, r):
        s_sb = SSB[it]
        v_sb = []
        for kg, (c0, m) in enumerate(KGS):
            vfb = pp.tile([m, 256], F32, tag="rtsm")
            MM(
                out=vfb[:], lhsT=kindT[:, c0 : c0 + m], rhs=FK[it][:],
                start=True, stop=True,
            )
            vkg = wp.tile([m, 256], F32, tag=f"v{kg}", bufs=3)
            nc.vector.tensor_mul(out=vkg[:].bitcast(F32R), in0=s_sb[kg][:], in1=vfb[:])
            v_sb.append(vkg)
        VS[it] = v_sb

    def vt_phase(it, r):
        v_sb = VS[it]
        vT = [
            wp.tile([128, 288], F32, tag=f"vT{sc}", name=f"vT{sc}_{it}_{r}", bufs=3)
            for sc in range(2)
        ]
        for kg, (c0, m) in enumerate(KGS):
            for sc in range(2):
                tp = pp.tile([128, m], F32, tag="tp")
                TP(
                    out=tp[:], in_=v_sb[kg][:, sc * 128 : (sc + 1) * 128],
                    identity=ident[0:m, 0:m],
                )
                nc.scalar.copy(out=vT[sc][:, c0 : c0 + m].bitcast(F32R), in_=tp[:])
        VT[it] = vT

    def agree_phase(it, r):
        pT = PT[it]
        vT = VT[it]
        b_t = BT[it]
        for ch in range(2):
            gps = pp.tile([128, 288], F32, tag="rtsm")
            for sc in range(2):
                MM(
                    out=gps[:], lhsT=pT[sc][:, ch * 128 : (ch + 1) * 128],
                    rhs=vT[sc][:], start=(sc == 0), stop=(sc == 1),
                )
            ga = wp.tile([128, 288], F32, tag=f"ga{ch}", bufs=3)
            nc.vector.tensor_mul(out=ga[:].bitcast(F32R), in0=wf[:, ch * 288 : (ch + 1) * 288].bitcast(F32), in1=gps[:])
            aps_ = pp.tile([4, 288], F32, tag="rtsm")
            MM(out=aps_[:], lhsT=uind[:], rhs=ga[:], start=True, stop=True)
            agr = sp.tile([4, 9], F32, tag=f"agr{ch}")
            nc.vector.reduce_sum(
                out=agr[:], in_=aps_[:].rearrange("p (k c) -> p k c", c=32),
                axis=AX.X,
            )
            if r == 0:
                nc.vector.tensor_copy(out=b_t[ch][:], in_=agr[:])
            else:
                nc.vector.tensor_add(out=b_t[ch][:], in0=b_t[ch][:], in1=agr[:])

    def emit_out(it):
        for kg, (c0, m) in enumerate(KGS):
            vm = sp.tile([m, 1], F32, tag=f"vm{kg}")
            nc.vector.reduce_sum(out=vm[:], in_=VS[it][kg][:], axis=AX.X)
            nc.sync.dma_start(
                out=out_ap[it, c0 : c0 + m].unsqueeze(1),
                in_=vm[:, 0:1],
            )

    for it in range(bl):
        a1(it)
    for it in range(bl + 1):
        if it < bl:
            a2(it)
        if it >= 1:
            a4(it - 1)
    for r in range(R):
        for it in range(bl + 1):
            if it < bl:
                if r > 0:
                    softmax_wcs(it)
                s_phase(it, r)
                sqk_phase(it, r)
            if it >= 1:
                v_phase(it - 1, r)
                if r < R - 1:
                    vt_phase(it - 1, r)
                    agree_phase(it - 1, r)
                else:
                    emit_out(it - 1)
    es.close()


def _pack_consts(inputs):
    conv1_w = np.ascontiguousarray(np.asarray(inputs["conv1_w"], np.float32))
    conv1_b = np.asarray(inputs["conv1_b"], np.float32)
    prim_w = np.ascontiguousarray(np.asarray(inputs["prim_w"], np.float32))
    prim_b = np.asarray(inputs["prim_b"], np.float32)
    W = np.asarray(inputs["W"], np.float32)

    w1 = np.zeros((128, 1152), np.float32)
    for t in range(9):
        for ec in range(4):
            w1[:, t * 128 + ec * 32 : t * 128 + (ec + 1) * 32] = conv1_w[
                :, ec * 128 : (ec + 1) * 128, t
            ].T
    w2 = np.zeros((32, 2304), np.float32)
    for t in range(9):
        w2[:, t * 256 : (t + 1) * 256] = prim_w[:, :, t].T
    wfr = W[0].transpose(0, 2, 1, 3).reshape(U, C, K * C)  # [u, c', (k c)]
    wf = np.zeros((128, 576), np.float32)
    for ch in range(2):
        wf[:, ch * 288 : (ch + 1) * 288] = wfr[ch * 4 : (ch + 1) * 4].reshape(128, 288)
    w9 = wf / 9.0
    b1 = conv1_b.reshape(32, 1).copy()
    b2 = prim_b.reshape(2, 128).T.copy()
    ident = np.eye(128, dtype=np.float32)

    ind_sq8 = np.zeros((128, 16), np.float32)
    for p in range(128):
        ind_sq8[p, p // 32] = 1.0
        ind_sq8[p, 12 + p // 32] = 1.0
    indT8 = np.zeros((8, 256), np.float32)
    for p in range(128):
        indT8[p // 32, p] = 1.0
        indT8[4 + p // 32, 128 + p] = 1.0
    kind = np.zeros((128, 36), np.float32)
    for kg in range(3):
        m = 128 if kg < 2 else 32
        for p in range(m):
            kind[p, kg * 12 + kg * 4 + p // 32] = 1.0
    kindT = np.zeros((12, 288), np.float32)
    for kg in range(3):
        m = 128 if kg < 2 else 32
        for p in range(m):
            kindT[kg * 4 + p // 32, kg * 128 + p] = 1.0
    uind = np.zeros((128, 4), np.float32)
    for p in range(128):
        uind[p, p // 32] = 1.0
    u4T = np.zeros((4, 128), np.float32)
    for p in range(128):
        u4T[p // 32, p] = 1.0

    ind36 = np.zeros((36, 256), np.float32)
    for ch in range(2):
        for p in range(128):
            ind36[ch * 32 + p // 32, ch * 128 + p] = 1.0

    fbc = np.zeros((128, 2), np.float32)
    fbc[:, 0] = 1e-8
    fbc[:, 1] = 1.0

    return {
        "w1": w1, "w2": w2, "wf": wf, "w9": w9, "b1": b1, "b2": b2,
        "ident": ident, "ind_sq8": ind_sq8, "indT8": indT8, "kind": kind,
        "kindT": kindT, "uind": uind, "u4T": u4T, "fb": fbc, "ind36": ind36,
    }


_NC_CACHE = {}


def build_nc(bl=BL):
    if bl in _NC_CACHE:
        return _NC_CACHE[bl]
    nc = bacc.Bacc(
        "TRN2", target_bir_lowering=False, debug=False, num_devices=NCORES
    )
    shapes = {
        "x": ([bl, L], I32), "mask": ([bl, L], F32), "emb": ([V, E], F32),
        "w1": ([128, 1152], F32R), "w2": ([32, 2304], F32R), "wf": ([128, 576], F32R),
        "w9": ([128, 576], F32R), "b1": ([32, 1], F32), "b2": ([128, 2], F32),
        "ident": ([128, 128], F32R), "ind_sq8": ([128, 16], F32R),
        "indT8": ([8, 256], F32R), "kind": ([128, 36], F32R),
        "kindT": ([12, 288], F32R), "uind": ([128, 4], F32R), "u4T": ([4, 128], F32R),
        "fb": ([128, 2], F32), "ind36": ([36, 256], F32R),
    }
    aps = {
        name: nc.dram_tensor(name, shp, dt, kind="ExternalInput").ap()
        for name, (shp, dt) in shapes.items()
    }
    aps["out"] = nc.dram_tensor("out", [bl, K * C], F32, kind="ExternalOutput").ap()
    with tile.TileContext(nc) as tc:
        _emit(tc, nc, aps, bl)
    nc.compile()
    _NC_CACHE[bl] = nc
    return nc


def make_in_maps(inputs, bl=BL, ncores=NCORES):
    consts = _pack_consts(inputs)
    x = np.ascontiguousarray(np.asarray(inputs["x"], np.int32).reshape(ncores, bl, L))
    mask = np.ascontiguousarray(
        np.asarray(inputs["attention_mask"], np.float32).reshape(ncores, bl, L)
    )
    emb = np.ascontiguousarray(np.asarray(inputs["emb"], np.float32))
    return [
        {"x": x[i], "mask": mask[i], "emb": emb, **consts} for i in range(ncores)
    ]


def kernel(x, attention_mask, emb, conv1_w, conv1_b, prim_w, prim_b, W):
    inputs = {
        "x": x, "attention_mask": attention_mask, "emb": emb,
        "conv1_w": conv1_w, "conv1_b": conv1_b,
        "prim_w": prim_w, "prim_b": prim_b, "W": W,
    }
    nc = build_nc(BL)
    in_maps = make_in_maps(inputs)
    res = run_bass_kernel_spmd(nc, in_maps, core_ids=list(range(NCORES)))
    out = np.concatenate(
        [res.results[i]["out"].reshape(BL, K, C) for i in range(NCORES)], axis=0
    )
    return out.astype(np.float32)


# revision 33
# speedup vs baseline: 1.5526x; 1.2815x over previous
"""CapsuleNet Trainium2 kernel.

Data-parallel over batch: 64 items -> 8 cores x 8 items. Weights replicated.

Math (per item), matching the reference:
  e   = emb[x] * mask                      [L=512, E=512]
  h   = relu(conv1d(e.T, k=9, pad=4) + b1) [C=32, L=512]
  p   = conv1d(h, k=9, pad=4, stride=2)+b2 [UC=256, S=256]
  p   = squash(p over C-blocks of 32)
  routing (R=3) with b (logits) independent of S:
    c[u,k] = softmax_k(b);  s[k] = sum_u c[u,k] * (W[u,k].T @ p_u)
    v[k] = squash_c(s[k]);  agree[u,k] = <W[u,k], p_u.T @ v[k]>;  b += agree
  out = mean_s(v)                          [K=9, C=32]

Key layout: everything channel-on-partitions, position-on-free:
  eT   [128e x 4, 520]  (4-col zero pad both sides for conv shifts)
  h    [32, 520]        (padded)
  p    [128 x 2, 256]   rows = (u_local*32 + c'), chunk ch = u//4
  s,v  [128, 256] x 3   rows = (k_local*32 + c), kgroups (k0-3, k4-7, k8)
u_hat is never materialized: s is computed straight from p with a
c-weighted W (lhsT), and agree via Gram matrices G = pT.T @ vT.
"""

import numpy as np

import concourse.bass as bass
import concourse.tile as tile
from concourse import bacc, mybir
from concourse.bass_utils import run_bass_kernel_spmd

F32 = mybir.dt.float32
I32 = mybir.dt.int32
AF = mybir.ActivationFunctionType
ALU = mybir.AluOpType
AX = mybir.AxisListType

V, E, L = 50000, 512, 512
B, U, C, K, R = 64, 8, 32, 9, 3
S = 256
NCORES = 8
BL = B // NCORES  # items per core
F32R = mybir.dt.float32r

# conv1 tap assignment to the 4 PE column groups (M=32 each)
TGROUPS = [[0, 4, 8], [1, 5], [2, 6], [3, 7]]
# routing k-groups: (cols in the 288-wide (k,c) axis, partition count)
KGS = [(0, 128), (128, 128), (256, 32)]


def _emit(tc, nc, aps, bl):
    from contextlib import ExitStack

    es = ExitStack()
    x_ap = aps["x"]
    mask_ap = aps["mask"]
    emb_ap = aps["emb"]
    out_ap = aps["out"]

    F32R = mybir.dt.float32r

    def MM(out, lhsT, rhs, **kw):
        return nc.tensor.matmul(
            out=out, lhsT=lhsT.bitcast(F32R), rhs=rhs.bitcast(F32R), **kw
        )

    def TP(out, in_, identity, **kw):
        return nc.tensor.transpose(
            out=out.bitcast(F32R), in_=in_.bitcast(F32R),
            identity=identity.bitcast(F32R), **kw
        )

    cp = es.enter_context(tc.tile_pool(name="consts", bufs=1))
    w1 = cp.tile([128, 1152], F32R)
    nc.sync.dma_start(out=w1[:], in_=aps["w1"])
    w2 = cp.tile([32, 2304], F32R)
    nc.sync.dma_start(out=w2[:], in_=aps["w2"])
    wf = cp.tile([128, 576], F32R)
    nc.sync.dma_start(out=wf[:], in_=aps["wf"])
    w9 = cp.tile([128, 576], F32R)
    nc.sync.dma_start(out=w9[:], in_=aps["w9"])
    b1 = cp.tile([32, 1], F32)
    nc.sync.dma_start(out=b1[:], in_=aps["b1"])
    b2 = cp.tile([128, 2], F32)
    nc.sync.dma_start(out=b2[:], in_=aps["b2"])
    ident = cp.tile([128, 128], F32R)
    nc.sync.dma_start(out=ident[:], in_=aps["ident"])
    ind_sq8 = cp.tile([128, 16], F32R)
    nc.sync.dma_start(out=ind_sq8[:], in_=aps["ind_sq8"])
    indT8 = cp.tile([8, 256], F32R)
    nc.sync.dma_start(out=indT8[:], in_=aps["indT8"])
    kind = cp.tile([128, 36], F32R)
    nc.sync.dma_start(out=kind[:], in_=aps["kind"])
    kindT = cp.tile([12, 288], F32R)
    nc.sync.dma_start(out=kindT[:], in_=aps["kindT"])
    uind = cp.tile([128, 4], F32R)
    nc.sync.dma_start(out=uind[:], in_=aps["uind"])
    u4T = cp.tile([4, 128], F32R)
    nc.sync.dma_start(out=u4T[:], in_=aps["u4T"])
    fb = cp.tile([128, 2], F32)  # col0 = 1e-8 (eps), col1 = 1.0
    nc.sync.dma_start(out=fb[:], in_=aps["fb"])

    gp = es.enter_context(tc.tile_pool(name="gather", bufs=2))
    wp = es.enter_context(tc.tile_pool(name="work", bufs=2))
    sp = es.enter_context(tc.tile_pool(name="small", bufs=2))
    pq = es.enter_context(tc.tile_pool(name="persist", bufs=1))
    pp = es.enter_context(tc.tile_pool(name="psum", bufs=2, space="PSUM"))

    HP = [None] * bl
    PSB = [None] * bl
    P2 = [None] * bl
    PSQ = [None] * bl
    F8 = [None] * bl
    PS = [None] * bl
    PT = [None] * bl
    BT = [None] * bl
    WCS = [None] * bl
    SSB = [None] * bl
    S2 = [None] * bl
    SQK = [None] * bl
    FK = [None] * bl
    VS = [None] * bl
    VT = [None] * bl

    # ---------- stage A1: gather, mask-diag transpose, conv1, collapse ----------
    def a1(it):
        eraws = []
        for lc in range(4):
            idx = gp.tile([128, 1], I32, tag="idx", bufs=2)
            nc.sync.dma_start(
                out=idx[:], in_=x_ap[it, lc * 128 : (lc + 1) * 128].unsqueeze(1)
            )
            msk = gp.tile([128, 1], F32, tag="msk", bufs=2)
            nc.sync.dma_start(
                out=msk[:],
                in_=mask_ap[it, lc * 128 : (lc + 1) * 128].unsqueeze(1),
            )
            eraw = gp.tile([128, 512], F32R, tag="eraw", bufs=4)
            nc.gpsimd.indirect_dma_start(
                out=eraw[:],
                out_offset=None,
                in_=emb_ap,
                in_offset=bass.IndirectOffsetOnAxis(ap=idx[:, 0:1], axis=0),
            )
            dmask = gp.tile([128, 128], F32, tag="dmask", bufs=4)
            nc.vector.tensor_scalar_mul(
                out=dmask[:].bitcast(F32R), in0=ident[:].bitcast(F32), scalar1=msk[:, 0:1]
            )
            eraws.append((eraw, dmask))
        eT = []
        for ec in range(4):
            t = gp.tile([128, 520], F32, tag=f"eT{ec}")
            nc.vector.memset(t[:, 0:4], 0.0)
            nc.vector.memset(t[:, 516:520], 0.0)
            eT.append(t)
        for lc in range(4):
            eraw, dmask = eraws[lc]
            for ec in range(4):
                ps = pp.tile([128, 128], F32, tag="tp")
                TP(
                    out=ps[:], in_=eraw[:, ec * 128 : (ec + 1) * 128],
                    identity=dmask[:],
                )
                nc.scalar.copy(
                    out=eT[ec][:, 4 + lc * 128 : 4 + (lc + 1) * 128].bitcast(F32R), in_=ps[:]
                )
        psy = pp.tile([32, 512], F32, tag="cva")
        cnt = 0
        for t in range(9):
            for ec in range(4):
                MM(
                    out=psy[:],
                    lhsT=w1[:, t * 128 + ec * 32 : t * 128 + (ec + 1) * 32],
                    rhs=eT[ec][:, t : t + 512],
                    start=(cnt == 0),
                    stop=(cnt == 35),
                )
                cnt += 1
        hp = pq.tile([32, 520], F32, tag=f"hp{it}", name=f"hp{it}")
        nc.vector.memset(hp[:, 0:4], 0.0)
        nc.vector.memset(hp[:, 516:520], 0.0)
        nc.scalar.activation(
            out=hp[:, 4:516].bitcast(F32R), in_=psy[:], func=AF.Relu, bias=b1[:, 0:1]
        )
        HP[it] = hp

    # ---------- stage A2: primary conv + bias + square ----------
    def a2(it):
        hp = HP[it]
        psp = pp.tile([128, 512], F32, tag="pra")
        for h in range(2):
            for t in range(9):
                rhs = hp[:, t : t + 512].rearrange("p (s two) -> p s two", two=2)[:, :, 0]
                MM(
                    out=psp[:, h * 256 : (h + 1) * 256],
                    lhsT=w2[:, t * 256 + h * 128 : t * 256 + (h + 1) * 128],
                    rhs=rhs,
                    start=(t == 0),
                    stop=(t == 8),
                )
        psb, p2 = [], []
        for h in range(2):
            sb = wp.tile([128, 256], F32, tag=f"psb{h}", bufs=3)
            nc.scalar.activation(
                out=sb[:], in_=psp[:, h * 256 : (h + 1) * 256], func=AF.Identity,
                bias=b2[:, h : h + 1],
            )
            psb.append(sb)
            q = wp.tile([128, 256], F32, tag=f"p2{h}", bufs=3)
            nc.scalar.square(out=q[:].bitcast(F32R), in_=sb[:])
            p2.append(q)
        PSB[it] = psb
        P2[it] = p2
        psq = pp.tile([8, 256], F32, tag="rtsm")
        MM(out=psq[:], lhsT=ind_sq8[:, 0:8], rhs=p2[0][:], start=True, stop=False)
        MM(out=psq[:], lhsT=ind_sq8[:, 8:16], rhs=p2[1][:], start=False, stop=True)
        t1 = sp.tile([8, 256], F32, tag="t1")
        nc.scalar.activation(out=t1[:], in_=psq[:], func=AF.Sqrt, bias=fb[0:8, 0:1])
        t2 = sp.tile([8, 256], F32, tag="t2")
        nc.scalar.activation(out=t2[:], in_=psq[:], func=AF.Identity, bias=fb[0:8, 1:2])
        t3 = sp.tile([8, 256], F32, tag="t3")
        nc.vector.tensor_mul(out=t3[:], in0=t1[:], in1=t2[:])
        t4 = sp.tile([8, 256], F32, tag="t4")
        nc.vector.reciprocal(out=t4[:], in_=t3[:])
        f8 = sp.tile([8, 256], F32, tag="f8", bufs=3)
        nc.vector.tensor_mul(out=f8[:].bitcast(F32R), in0=psq[:], in1=t4[:])
        F8[it] = f8

    # ---------- stage A4: broadcast, squash-mult, transpose p ----------
    def a4(it):
        psb = PSB[it]
        ps_t, pT = [], []
        for h in range(2):
            pfb = pp.tile([128, 256], F32, tag="rtsm")
            MM(
                out=pfb[:], lhsT=indT8[:, h * 128 : (h + 1) * 128], rhs=F8[it][:],
                start=True, stop=True,
            )
            pst = pq.tile([128, 256], F32, tag=f"ps{h}_{it}", name=f"ps{h}_{it}")
            nc.vector.tensor_mul(out=pst[:].bitcast(F32R), in0=psb[h][:], in1=pfb[:])
            ps_t.append(pst)
        for sc in range(2):
            t = pq.tile([128, 256], F32, tag=f"pT{sc}_{it}", name=f"pT{sc}_{it}")
            pT.append(t)
        for h in range(2):
            for sc in range(2):
                tp = pp.tile([128, 128], F32, tag="tp")
                TP(
                    out=tp[:], in_=ps_t[h][:, sc * 128 : (sc + 1) * 128],
                    identity=ident[:],
                )
                nc.scalar.copy(out=pT[sc][:, h * 128 : (h + 1) * 128].bitcast(F32R), in_=tp[:])
        PS[it] = ps_t
        PT[it] = pT
        BT[it] = [
            pq.tile([4, 9], F32, tag=f"bt{ch}_{it}", name=f"bt{ch}_{it}")
            for ch in range(2)
        ]

    # ---------- routing phases ----------
    def softmax_wcs(it):
        b_t = BT[it]
        wcs = []
        for ch in range(2):
            negm = sp.tile([4, 1], F32, tag="negm")
            nc.vector.reduce_max(out=negm[:], in_=b_t[ch][:], axis=AX.X, negate=True)
            ex = sp.tile([4, 9], F32, tag="ex")
            nc.scalar.activation(out=ex[:], in_=b_t[ch][:], func=AF.Exp, bias=negm[:, 0:1])
            sm = sp.tile([4, 1], F32, tag="sm9")
            nc.vector.reduce_sum(out=sm[:], in_=ex[:], axis=AX.X)
            rs = sp.tile([4, 1], F32, tag="rs")
            nc.vector.reciprocal(out=rs[:], in_=sm[:])
            cc = sp.tile([4, 9], F32, tag="cc")
            nc.vector.tensor_scalar_mul(out=cc[:], in0=ex[:], scalar1=rs[:, 0:1])
            csm = sp.tile([4, 288], F32, tag="csm")
            nc.vector.tensor_copy(
                out=csm[:].rearrange("p (k c) -> p k c", c=32).bitcast(F32R),
                in_=cc[:].unsqueeze(2).to_broadcast([4, 9, 32]),
            )
            sbc = pp.tile([128, 288], F32, tag="rtsm")
            MM(out=sbc[:], lhsT=u4T[:], rhs=csm[:], start=True, stop=True)
            wc = wp.tile([128, 288], F32, tag=f"wcs{ch}", bufs=3)
            nc.vector.tensor_mul(out=wc[:].bitcast(F32R), in0=wf[:, ch * 288 : (ch + 1) * 288].bitcast(F32), in1=sbc[:])
            wcs.append(wc)
        WCS[it] = wcs

    def s_phase(it, r):
        ps_t = PS[it]

        def lhs_s(ch, c0, c1):
            if r == 0:
                return w9[:, ch * 288 + c0 : ch * 288 + c1]
            return WCS[it][ch][:, c0:c1]

        s_sb, s2 = [], []
        for kg, (c0, m) in enumerate(KGS):
            sps = pp.tile([m, 256], F32, tag="rtsm")
            for ch in range(2):
                MM(
                    out=sps[:], lhsT=lhs_s(ch, c0, c0 + m), rhs=ps_t[ch][:],
                    start=(ch == 0), stop=(ch == 1),
                )
            ssb = wp.tile([m, 256], F32, tag=f"ssb{kg}", bufs=3)
            nc.scalar.copy(out=ssb[:], in_=sps[:])
            s_sb.append(ssb)
            q = wp.tile([m, 256], F32, tag=f"s2{kg}", bufs=3)
            nc.scalar.square(out=q[:].bitcast(F32R), in_=sps[:])
            s2.append(q)
        SSB[it] = s_sb
        S2[it] = s2

    def sqk_phase(it, r):
        s2 = S2[it]
        sqk = pp.tile([12, 256], F32, tag="rtsm")
        for kg, (c0, m) in enumerate(KGS):
            MM(
                out=sqk[:], lhsT=kind[0:m, kg * 12 : (kg + 1) * 12], rhs=s2[kg][:],
                start=(kg == 0), stop=(kg == 2),
            )
        u1 = sp.tile([12, 256], F32, tag="u1")
        nc.scalar.activation(out=u1[:], in_=sqk[:], func=AF.Sqrt, bias=fb[0:12, 0:1])
        u2 = sp.tile([12, 256], F32, tag="u2")
        nc.scalar.activation(out=u2[:], in_=sqk[:], func=AF.Identity, bias=fb[0:12, 1:2])
        u3 = sp.tile([12, 256], F32, tag="u3")
        nc.vector.tensor_mul(out=u3[:], in0=u1[:], in1=u2[:])
        u4 = sp.tile([12, 256], F32, tag="u4")
        nc.vector.reciprocal(out=u4[:], in_=u3[:])
        fk = sp.tile([12, 256], F32, tag="fk", bufs=3)
        nc.vector.tensor_mul(out=fk[:].bitcast(F32R), in0=sqk[:], in1=u4[:])
        if r == R - 1:
            fks = sp.tile([12, 256], F32, tag="fks", bufs=3)
            nc.scalar.mul(out=fks[:].bitcast(F32R), in_=fk[:].bitcast(F32), mul=1.0 / S)
            fk = fks
        FK[it] = fk

    def v_phase(it, r):
        s_sb = SSB[it]
        v_sb = []
        for kg, (c0, m) in enumerate(KGS):
            vfb = pp.tile([m, 256], F32, tag="rtsm")
            MM(
                out=vfb[:], lhsT=kindT[:, c0 : c0 + m], rhs=FK[it][:],
                start=True, stop=True,
            )
            vkg = wp.tile([m, 256], F32, tag=f"v{kg}", bufs=3)
            nc.vector.tensor_mul(out=vkg[:].bitcast(F32R), in0=s_sb[kg][:], in1=vfb[:])
            v_sb.append(vkg)
        VS[it] = v_sb

    def vt_phase(it, r):
        v_sb = VS[it]
        vT = [
            wp.tile([128, 288], F32, tag=f"vT{sc}", name=f"vT{sc}_{it}_{r}", bufs=3)
            for sc in range(2)
        ]
        for kg, (c0, m) in enumerate(KGS):
            for sc in range(2):
                tp = pp.tile([128, m], F32, tag="tp")
                TP(
                    out=tp[:], in_=v_sb[kg][:, sc * 128 : (sc + 1) * 128],
                    identity=ident[0:m, 0:m],
                )
                nc.scalar.copy(out=vT[sc][:, c0 : c0 + m].bitcast(F32R), in_=tp[:])
        VT[it] = vT

    def agree_phase(it, r):
        pT = PT[it]
        vT = VT[it]
        b_t = BT[it]
        for ch in range(2):
            gps = pp.tile([128, 288], F32, tag="rtsm")
            for sc in range(2):
                MM(
                    out=gps[:], lhsT=pT[sc][:, ch * 128 : (ch + 1) * 128],
                    rhs=vT[sc][:], start=(sc == 0), stop=(sc == 1),
                )
            ga = wp.tile([128, 288], F32, tag=f"ga{ch}", bufs=3)
            nc.vector.tensor_mul(out=ga[:].bitcast(F32R), in0=wf[:, ch * 288 : (ch + 1) * 288].bitcast(F32), in1=gps[:])
            aps_ = pp.tile([4, 288], F32, tag="rtsm")
            MM(out=aps_[:], lhsT=uind[:], rhs=ga[:], start=True, stop=True)
            agr = sp.tile([4, 9], F32, tag=f"agr{ch}")
            nc.vector.reduce_sum(
                out=agr[:], in_=aps_[:].rearrange("p (k c) -> p k c", c=32),
                axis=AX.X,
            )
            if r == 0:
                nc.vector.tensor_copy(out=b_t[ch][:], in_=agr[:])
            else:
                nc.vector.tensor_add(out=b_t[ch][:], in0=b_t[ch][:], in1=agr[:])

    def emit_out(it):
        for kg, (c0, m) in enumerate(KGS):
            vm = sp.tile([m, 1], F32, tag=f"vm{kg}")
            nc.vector.reduce_sum(out=vm[:], in_=VS[it][kg][:], axis=AX.X)
            nc.sync.dma_start(
                out=out_ap[it, c0 : c0 + m].unsqueeze(1),
                in_=vm[:, 0:1],
            )

    for it in range(bl):
        a1(it)
    for it in range(bl + 1):
        if it < bl:
            a2(it)
        if it >= 1:
            a4(it - 1)
    for r in range(R):
        for it in range(bl + 1):
            if it < bl:
                if r > 0:
                    softmax_wcs(it)
                s_phase(it, r)
                sqk_phase(it, r)
            if it >= 1:
                v_phase(it - 1, r)
                if r < R - 1:
                    vt_phase(it - 1, r)
                    agree_phase(it - 1, r)
                else:
                    emit_out(it - 1)
    es.close()


def _pack_consts(inputs):
    conv1_w = np.ascontiguousarray(np.asarray(inputs["conv1_w"], np.float32))
    conv1_b = np.asarray(inputs["conv1_b"], np.float32)
    prim_w = np.ascontiguousarray(np.asarray(inputs["prim_w"], np.float32))
    prim_b = np.asarray(inputs["prim_b"], np.float32)
    W = np.asarray(inputs["W"], np.float32)

    w1 = np.zeros((128, 1152), np.float32)
    for t in range(9):
        for ec in range(4):
            w1[:, t * 128 + ec * 32 : t * 128 + (ec + 1) * 32] = conv1_w[
                :, ec * 128 : (ec + 1) * 128, t
            ].T
    w2 = np.zeros((32, 2304), np.float32)
    for t in range(9):
        w2[:, t * 256 : (t + 1) * 256] = prim_w[:, :, t].T
    wfr = W[0].transpose(0, 2, 1, 3).reshape(U, C, K * C)  # [u, c', (k c)]
    wf = np.zeros((128, 576), np.float32)
    for ch in range(2):
        wf[:, ch * 288 : (ch + 1) * 288] = wfr[ch * 4 : (ch + 1) * 4].reshape(128, 288)
    w9 = wf / 9.0
    b1 = conv1_b.reshape(32, 1).copy()
    b2 = prim_b.reshape(2, 128).T.copy()
    ident = np.eye(128, dtype=np.float32)

    ind_sq8 = np.zeros((128, 16), np.float32)
    for p in range(128):
        ind_sq8[p, p // 32] = 1.0
        ind_sq8[p, 12 + p // 32] = 1.0
    indT8 = np.zeros((8, 256), np.float32)
    for p in range(128):
        indT8[p // 32, p] = 1.0
        indT8[4 + p // 32, 128 + p] = 1.0
    kind = np.zeros((128, 36), np.float32)
    for kg in range(3):
        m = 128 if kg < 2 else 32
        for p in range(m):
            kind[p, kg * 12 + kg * 4 + p // 32] = 1.0
    kindT = np.zeros((12, 288), np.float32)
    for kg in range(3):
        m = 128 if kg < 2 else 32
        for p in range(m):
            kindT[kg * 4 + p // 32, kg * 128 + p] = 1.0
    uind = np.zeros((128, 4), np.float32)
    for p in range(128):
        uind[p, p // 32] = 1.0
    u4T = np.zeros((4, 128), np.float32)
    for p in range(128):
        u4T[p // 32, p] = 1.0

    ind36 = np.zeros((36, 256), np.float32)
    for ch in range(2):
        for p in range(128):
            ind36[ch * 32 + p // 32, ch * 128 + p] = 1.0

    fbc = np.zeros((128, 2), np.float32)
    fbc[:, 0] = 1e-8
    fbc[:, 1] = 1.0

    return {
        "w1": w1, "w2": w2, "wf": wf, "w9": w9, "b1": b1, "b2": b2,
        "ident": ident, "ind_sq8": ind_sq8, "indT8": indT8, "kind": kind,
        "kindT": kindT, "uind": uind, "u4T": u4T, "fb": fbc, "ind36": ind36,
    }


_NC_CACHE = {}


def build_nc(bl=BL):
    if bl in _NC_CACHE:
        return _NC_CACHE[bl]
    nc = bacc.Bacc(
        "TRN2", target_bir_lowering=False, debug=False, num_devices=NCORES
    )
    shapes = {
        "x": ([bl, L], I32), "mask": ([bl, L], F32), "emb": ([V, E], F32),
        "w1": ([128, 1152], F32R), "w2": ([32, 2304], F32R), "wf": ([128, 576], F32R),
        "w9": ([128, 576], F32R), "b1": ([32, 1], F32), "b2": ([128, 2], F32),
        "ident": ([128, 128], F32R), "ind_sq8": ([128, 16], F32R),
        "indT8": ([8, 256], F32R), "kind": ([128, 36], F32R),
        "kindT": ([12, 288], F32R), "uind": ([128, 4], F32R), "u4T": ([4, 128], F32R),
        "fb": ([128, 2], F32), "ind36": ([36, 256], F32R),
    }
    aps = {
        name: nc.dram_tensor(name, shp, dt, kind="ExternalInput").ap()
        for name, (shp, dt) in shapes.items()
    }
    aps["out"] = nc.dram_tensor("out", [bl, K * C], F32, kind="ExternalOutput").ap()
    with tile.TileContext(nc) as tc:
        _emit(tc, nc, aps, bl)
    nc.compile()
    _NC_CACHE[bl] = nc
    return nc


def make_in_maps(inputs, bl=BL, ncores=NCORES):
    consts = _pack_consts(inputs)
    x = np.ascontiguousarray(np.asarray(inputs["x"], np.int32).reshape(ncores, bl, L))
    mask = np.ascontiguousarray(
        np.asarray(inputs["attention_mask"], np.float32).reshape(ncores, bl, L)
    )
    emb = np.ascontiguousarray(np.asarray(inputs["emb"], np.float32))
    return [
        {"x": x[i], "mask": mask[i], "emb": emb, **consts} for i in range(ncores)
    ]


def kernel(x, attention_mask, emb, conv1_w, conv1_b, prim_w, prim_b, W):
    inputs = {
        "x": x, "attention_mask": attention_mask, "emb": emb,
        "conv1_w": conv1_w, "conv1_b": conv1_b,
        "prim_w": prim_w, "prim_b": prim_b, "W": W,
    }
    nc = build_nc(BL)
    in_maps = make_in_maps(inputs)
    res = run_bass_kernel_spmd(nc, in_maps, core_ids=list(range(NCORES)))
    out = np.concatenate(
        [res.results[i]["out"].reshape(BL, K, C) for i in range(NCORES)], axis=0
    )
    return out.astype(np.float32)


# revision 34
# speedup vs baseline: 1.6338x; 1.0523x over previous
"""CapsuleNet Trainium2 kernel.

Data-parallel over batch: 64 items -> 8 cores x 8 items. Weights replicated.

Math (per item), matching the reference:
  e   = emb[x] * mask                      [L=512, E=512]
  h   = relu(conv1d(e.T, k=9, pad=4) + b1) [C=32, L=512]
  p   = conv1d(h, k=9, pad=4, stride=2)+b2 [UC=256, S=256]
  p   = squash(p over C-blocks of 32)
  routing (R=3) with b (logits) independent of S:
    c[u,k] = softmax_k(b);  s[k] = sum_u c[u,k] * (W[u,k].T @ p_u)
    v[k] = squash_c(s[k]);  agree[u,k] = <W[u,k], p_u.T @ v[k]>;  b += agree
  out = mean_s(v)                          [K=9, C=32]

Key layout: everything channel-on-partitions, position-on-free:
  eT   [128e x 4, 520]  (4-col zero pad both sides for conv shifts)
  h    [32, 520]        (padded)
  p    [128 x 2, 256]   rows = (u_local*32 + c'), chunk ch = u//4
  s,v  [128, 256] x 3   rows = (k_local*32 + c), kgroups (k0-3, k4-7, k8)
u_hat is never materialized: s is computed straight from p with a
c-weighted W (lhsT), and agree via Gram matrices G = pT.T @ vT.
"""

import numpy as np

import concourse.bass as bass
import concourse.tile as tile
from concourse import bacc, mybir
from concourse.bass_utils import run_bass_kernel_spmd

F32 = mybir.dt.float32
I32 = mybir.dt.int32
AF = mybir.ActivationFunctionType
ALU = mybir.AluOpType
AX = mybir.AxisListType

V, E, L = 50000, 512, 512
B, U, C, K, R = 64, 8, 32, 9, 3
S = 256
NCORES = 8
BL = B // NCORES  # items per core
F32R = mybir.dt.float32r

# conv1 tap assignment to the 4 PE column groups (M=32 each)
TGROUPS = [[0, 4, 8], [1, 5], [2, 6], [3, 7]]
# routing k-groups: (cols in the 288-wide (k,c) axis, partition count)
KGS = [(0, 128), (128, 128), (256, 32)]


def _emit(tc, nc, aps, bl):
    from contextlib import ExitStack

    es = ExitStack()
    x_ap = aps["x"]
    mask_ap = aps["mask"]
    emb_ap = aps["emb"]
    out_ap = aps["out"]

    F32R = mybir.dt.float32r

    def MM(out, lhsT, rhs, **kw):
        return nc.tensor.matmul(
            out=out, lhsT=lhsT.bitcast(F32R), rhs=rhs.bitcast(F32R), **kw
        )

    def TP(out, in_, identity, **kw):
        return nc.tensor.transpose(
            out=out.bitcast(F32R), in_=in_.bitcast(F32R),
            identity=identity.bitcast(F32R), **kw
        )

    cp = es.enter_context(tc.tile_pool(name="consts", bufs=1))
    w1 = cp.tile([128, 1152], F32R)
    nc.sync.dma_start(out=w1[:], in_=aps["w1"])
    w2 = cp.tile([32, 2304], F32R)
    nc.sync.dma_start(out=w2[:], in_=aps["w2"])
    wf = cp.tile([128, 576], F32R)
    nc.sync.dma_start(out=wf[:], in_=aps["wf"])
    w9 = cp.tile([128, 576], F32R)
    nc.sync.dma_start(out=w9[:], in_=aps["w9"])
    b1 = cp.tile([32, 1], F32)
    nc.sync.dma_start(out=b1[:], in_=aps["b1"])
    b2 = cp.tile([128, 2], F32)
    nc.sync.dma_start(out=b2[:], in_=aps["b2"])
    ident = cp.tile([128, 128], F32R)
    nc.sync.dma_start(out=ident[:], in_=aps["ident"])
    ind_sq8 = cp.tile([128, 16], F32R)
    nc.sync.dma_start(out=ind_sq8[:], in_=aps["ind_sq8"])
    indT8 = cp.tile([8, 256], F32R)
    nc.sync.dma_start(out=indT8[:], in_=aps["indT8"])
    kind = cp.tile([128, 36], F32R)
    nc.sync.dma_start(out=kind[:], in_=aps["kind"])
    kindT = cp.tile([12, 288], F32R)
    nc.sync.dma_start(out=kindT[:], in_=aps["kindT"])
    uind = cp.tile([128, 4], F32R)
    nc.sync.dma_start(out=uind[:], in_=aps["uind"])
    u4T = cp.tile([4, 128], F32R)
    nc.sync.dma_start(out=u4T[:], in_=aps["u4T"])
    fb = cp.tile([128, 2], F32)  # col0 = 1e-8 (eps), col1 = 1.0
    nc.sync.dma_start(out=fb[:], in_=aps["fb"])

    gp = es.enter_context(tc.tile_pool(name="gather", bufs=2))
    wp = es.enter_context(tc.tile_pool(name="work", bufs=2))
    sp = es.enter_context(tc.tile_pool(name="small", bufs=2))
    pq = es.enter_context(tc.tile_pool(name="persist", bufs=1))
    pp = es.enter_context(tc.tile_pool(name="psum", bufs=2, space="PSUM"))

    HP = [None] * bl
    PSB = [None] * bl
    P2 = [None] * bl
    PSQ = [None] * bl
    F8 = [None] * bl
    PS = [None] * bl
    PT = [None] * bl
    BT = [None] * bl
    WCS = [None] * bl
    SSB = [None] * bl
    S2 = [None] * bl
    SQK = [None] * bl
    FK = [None] * bl
    VS = [None] * bl
    VT = [None] * bl

    # ---------- stage A1: gather, mask-diag transpose, conv1, collapse ----------
    def a1(it):
        eraws = []
        for lc in range(4):
            idx = gp.tile([128, 1], I32, tag="idx", bufs=2)
            nc.sync.dma_start(
                out=idx[:], in_=x_ap[it, lc * 128 : (lc + 1) * 128].unsqueeze(1)
            )
            msk = gp.tile([128, 1], F32, tag="msk", bufs=2)
            nc.sync.dma_start(
                out=msk[:],
                in_=mask_ap[it, lc * 128 : (lc + 1) * 128].unsqueeze(1),
            )
            eraw = gp.tile([128, 512], F32R, tag="eraw", bufs=4)
            nc.gpsimd.indirect_dma_start(
                out=eraw[:],
                out_offset=None,
                in_=emb_ap,
                in_offset=bass.IndirectOffsetOnAxis(ap=idx[:, 0:1], axis=0),
            )
            dmask = gp.tile([128, 128], F32, tag="dmask", bufs=4)
            nc.vector.tensor_scalar_mul(
                out=dmask[:].bitcast(F32R), in0=ident[:].bitcast(F32), scalar1=msk[:, 0:1]
            )
            eraws.append((eraw, dmask))
        eT = []
        for ec in range(4):
            t = gp.tile([128, 520], F32, tag=f"eT{ec}")
            nc.vector.memset(t[:, 0:4], 0.0)
            nc.vector.memset(t[:, 516:520], 0.0)
            eT.append(t)
        for lc in range(4):
            eraw, dmask = eraws[lc]
            for ec in range(4):
                ps = pp.tile([128, 128], F32, tag="tp")
                TP(
                    out=ps[:], in_=eraw[:, ec * 128 : (ec + 1) * 128],
                    identity=dmask[:],
                )
                nc.scalar.copy(
                    out=eT[ec][:, 4 + lc * 128 : 4 + (lc + 1) * 128].bitcast(F32R), in_=ps[:]
                )
        psy = pp.tile([32, 512], F32, tag="cva")
        cnt = 0
        for t in range(9):
            for ec in range(4):
                MM(
                    out=psy[:],
                    lhsT=w1[:, t * 128 + ec * 32 : t * 128 + (ec + 1) * 32],
                    rhs=eT[ec][:, t : t + 512],
                    start=(cnt == 0),
                    stop=(cnt == 35),
                )
                cnt += 1
        hp = pq.tile([32, 520], F32, tag=f"hp{it}", name=f"hp{it}")
        nc.vector.memset(hp[:, 0:4], 0.0)
        nc.vector.memset(hp[:, 516:520], 0.0)
        nc.scalar.activation(
            out=hp[:, 4:516].bitcast(F32R), in_=psy[:], func=AF.Relu, bias=b1[:, 0:1]
        )
        HP[it] = hp

    # ---------- stage A2: primary conv + bias + square ----------
    def a2(it):
        hp = HP[it]
        psp = pp.tile([128, 512], F32, tag="pra")
        for h in range(2):
            for t in range(9):
                rhs = hp[:, t : t + 512].rearrange("p (s two) -> p s two", two=2)[:, :, 0]
                MM(
                    out=psp[:, h * 256 : (h + 1) * 256],
                    lhsT=w2[:, t * 256 + h * 128 : t * 256 + (h + 1) * 128],
                    rhs=rhs,
                    start=(t == 0),
                    stop=(t == 8),
                )
        psb, p2 = [], []
        for h in range(2):
            sb = wp.tile([128, 256], F32, tag=f"psb{h}", bufs=3)
            nc.scalar.activation(
                out=sb[:], in_=psp[:, h * 256 : (h + 1) * 256], func=AF.Identity,
                bias=b2[:, h : h + 1],
            )
            psb.append(sb)
            q = wp.tile([128, 256], F32, tag=f"p2{h}", bufs=3)
            nc.scalar.square(out=q[:].bitcast(F32R), in_=sb[:])
            p2.append(q)
        PSB[it] = psb
        P2[it] = p2
        psq = pp.tile([8, 256], F32, tag="rtsm")
        MM(out=psq[:], lhsT=ind_sq8[:, 0:8], rhs=p2[0][:], start=True, stop=False)
        MM(out=psq[:], lhsT=ind_sq8[:, 8:16], rhs=p2[1][:], start=False, stop=True)
        t1 = sp.tile([8, 256], F32, tag="t1")
        nc.scalar.activation(out=t1[:], in_=psq[:], func=AF.Sqrt, bias=fb[0:8, 0:1])
        t2 = sp.tile([8, 256], F32, tag="t2")
        nc.scalar.activation(out=t2[:], in_=psq[:], func=AF.Identity, bias=fb[0:8, 1:2])
        t3 = sp.tile([8, 256], F32, tag="t3")
        nc.vector.tensor_mul(out=t3[:], in0=t1[:], in1=t2[:])
        t4 = sp.tile([8, 256], F32, tag="t4")
        nc.vector.reciprocal(out=t4[:], in_=t3[:])
        f8 = sp.tile([8, 256], F32, tag="f8", bufs=3)
        nc.vector.tensor_mul(out=f8[:].bitcast(F32R), in0=psq[:], in1=t4[:])
        F8[it] = f8

    # ---------- stage A4: broadcast, squash-mult, transpose p ----------
    def a4(it):
        psb = PSB[it]
        ps_t, pT = [], []
        for h in range(2):
            pfb = pp.tile([128, 256], F32, tag="rtsm")
            MM(
                out=pfb[:], lhsT=indT8[:, h * 128 : (h + 1) * 128], rhs=F8[it][:],
                start=True, stop=True,
            )
            pst = pq.tile([128, 256], F32, tag=f"ps{h}_{it}", name=f"ps{h}_{it}")
            nc.vector.tensor_mul(out=pst[:].bitcast(F32R), in0=psb[h][:], in1=pfb[:])
            ps_t.append(pst)
        for sc in range(2):
            t = pq.tile([128, 256], F32, tag=f"pT{sc}_{it}", name=f"pT{sc}_{it}")
            pT.append(t)
        for h in range(2):
            for sc in range(2):
                tp = pp.tile([128, 128], F32, tag="tp")
                TP(
                    out=tp[:], in_=ps_t[h][:, sc * 128 : (sc + 1) * 128],
                    identity=ident[:],
                )
                nc.scalar.copy(out=pT[sc][:, h * 128 : (h + 1) * 128].bitcast(F32R), in_=tp[:])
        PS[it] = ps_t
        PT[it] = pT
        BT[it] = [
            pq.tile([4, 9], F32, tag=f"bt{ch}_{it}", name=f"bt{ch}_{it}")
            for ch in range(2)
        ]

    # ---------- routing phases ----------
    def softmax_wcs(it):
        b_t = BT[it]
        wcs = []
        for ch in range(2):
            negm = sp.tile([4, 1], F32, tag="negm")
            nc.vector.reduce_max(out=negm[:], in_=b_t[ch][:], axis=AX.X, negate=True)
            ex = sp.tile([4, 9], F32, tag="ex")
            nc.scalar.activation(out=ex[:], in_=b_t[ch][:], func=AF.Exp, bias=negm[:, 0:1])
            sm = sp.tile([4, 1], F32, tag="sm9")
            nc.vector.reduce_sum(out=sm[:], in_=ex[:], axis=AX.X)
            rs = sp.tile([4, 1], F32, tag="rs")
            nc.vector.reciprocal(out=rs[:], in_=sm[:])
            cc = sp.tile([4, 9], F32, tag="cc")
            nc.vector.tensor_scalar_mul(out=cc[:], in0=ex[:], scalar1=rs[:, 0:1])
            csm = sp.tile([4, 288], F32, tag="csm")
            nc.vector.tensor_copy(
                out=csm[:].rearrange("p (k c) -> p k c", c=32).bitcast(F32R),
                in_=cc[:].unsqueeze(2).to_broadcast([4, 9, 32]),
            )
            sbc = pp.tile([128, 288], F32, tag="rtsm")
            MM(out=sbc[:], lhsT=u4T[:], rhs=csm[:], start=True, stop=True)
            wc = wp.tile([128, 288], F32, tag=f"wcs{ch}", bufs=3)
            nc.vector.tensor_mul(out=wc[:].bitcast(F32R), in0=wf[:, ch * 288 : (ch + 1) * 288].bitcast(F32), in1=sbc[:])
            wcs.append(wc)
        WCS[it] = wcs

    def s_phase(it, r):
        ps_t = PS[it]

        def lhs_s(ch, c0, c1):
            if r == 0:
                return w9[:, ch * 288 + c0 : ch * 288 + c1]
            return WCS[it][ch][:, c0:c1]

        s_sb, s2 = [], []
        for kg, (c0, m) in enumerate(KGS):
            sps = pp.tile([m, 256], F32, tag="rtsm")
            for ch in range(2):
                MM(
                    out=sps[:], lhsT=lhs_s(ch, c0, c0 + m), rhs=ps_t[ch][:],
                    start=(ch == 0), stop=(ch == 1),
                )
            ssb = wp.tile([m, 256], F32, tag=f"ssb{kg}", bufs=3)
            nc.scalar.copy(out=ssb[:], in_=sps[:])
            s_sb.append(ssb)
            q = wp.tile([m, 256], F32, tag=f"s2{kg}", bufs=3)
            nc.scalar.square(out=q[:].bitcast(F32R), in_=sps[:])
            s2.append(q)
        SSB[it] = s_sb
        S2[it] = s2

    def sqk_phase(it, r):
        s2 = S2[it]
        sqk = pp.tile([12, 256], F32, tag="rtsm")
        for kg, (c0, m) in enumerate(KGS):
            MM(
                out=sqk[:], lhsT=kind[0:m, kg * 12 : (kg + 1) * 12], rhs=s2[kg][:],
                start=(kg == 0), stop=(kg == 2),
            )
        u1 = sp.tile([12, 256], F32, tag="u1")
        nc.scalar.activation(out=u1[:], in_=sqk[:], func=AF.Sqrt, bias=fb[0:12, 0:1])
        u2 = sp.tile([12, 256], F32, tag="u2")
        nc.scalar.activation(out=u2[:], in_=sqk[:], func=AF.Identity, bias=fb[0:12, 1:2])
        u3 = sp.tile([12, 256], F32, tag="u3")
        nc.vector.tensor_mul(out=u3[:], in0=u1[:], in1=u2[:])
        u4 = sp.tile([12, 256], F32, tag="u4")
        nc.vector.reciprocal(out=u4[:], in_=u3[:])
        fk = sp.tile([12, 256], F32, tag="fk", bufs=3)
        nc.vector.tensor_mul(out=fk[:].bitcast(F32R), in0=sqk[:], in1=u4[:])
        if r == R - 1:
            fks = sp.tile([12, 256], F32, tag="fks", bufs=3)
            nc.scalar.mul(out=fks[:].bitcast(F32R), in_=fk[:].bitcast(F32), mul=1.0 / S)
            fk = fks
        FK[it] = fk

    def v_phase(it, r):
        s_sb = SSB[it]
        v_sb = []
        for kg, (c0, m) in enumerate(KGS):
            vfb = pp.tile([m, 256], F32, tag="rtsm")
            MM(
                out=vfb[:], lhsT=kindT[:, c0 : c0 + m], rhs=FK[it][:],
                start=True, stop=True,
            )
            vkg = wp.tile([m, 256], F32, tag=f"v{kg}", bufs=3)
            nc.vector.tensor_mul(out=vkg[:].bitcast(F32R), in0=s_sb[kg][:], in1=vfb[:])
            v_sb.append(vkg)
        VS[it] = v_sb

    def vt_phase(it, r):
        v_sb = VS[it]
        vT = [
            wp.tile([128, 288], F32, tag=f"vT{sc}", name=f"vT{sc}_{it}_{r}", bufs=3)
            for sc in range(2)
        ]
        for kg, (c0, m) in enumerate(KGS):
            for sc in range(2):
                tp = pp.tile([128, m], F32, tag="tp")
                TP(
                    out=tp[:], in_=v_sb[kg][:, sc * 128 : (sc + 1) * 128],
                    identity=ident[0:m, 0:m],
                )
                nc.scalar.copy(out=vT[sc][:, c0 : c0 + m].bitcast(F32R), in_=tp[:])
        VT[it] = vT

    def agree_phase(it, r):
        pT = PT[it]
        vT = VT[it]
        b_t = BT[it]
        for ch in range(2):
            gps = pp.tile([128, 288], F32, tag="rtsm")
            for sc in range(2):
                MM(
                    out=gps[:], lhsT=pT[sc][:, ch * 128 : (ch + 1) * 128],
                    rhs=vT[sc][:], start=(sc == 0), stop=(sc == 1),
                )
            ga = wp.tile([128, 288], F32, tag=f"ga{ch}", bufs=3)
            nc.vector.tensor_mul(out=ga[:].bitcast(F32R), in0=wf[:, ch * 288 : (ch + 1) * 288].bitcast(F32), in1=gps[:])
            aps_ = pp.tile([4, 288], F32, tag="rtsm")
            MM(out=aps_[:], lhsT=uind[:], rhs=ga[:], start=True, stop=True)
            agr = sp.tile([4, 9], F32, tag=f"agr{ch}")
            nc.vector.reduce_sum(
                out=agr[:], in_=aps_[:].rearrange("p (k c) -> p k c", c=32),
                axis=AX.X,
            )
            if r == 0:
                nc.vector.tensor_copy(out=b_t[ch][:], in_=agr[:])
            else:
                nc.vector.tensor_add(out=b_t[ch][:], in0=b_t[ch][:], in1=agr[:])

    def emit_out(it):
        for kg, (c0, m) in enumerate(KGS):
            vm = sp.tile([m, 1], F32, tag=f"vm{kg}")
            nc.vector.reduce_sum(out=vm[:], in_=VS[it][kg][:], axis=AX.X)
            nc.sync.dma_start(
                out=out_ap[it, c0 : c0 + m].unsqueeze(1),
                in_=vm[:, 0:1],
            )

    def stage(it, st):
        if st == 0:
            a1(it)
        elif st == 1:
            a2(it)
        elif st == 2:
            a4(it)
        elif st in (3, 5, 7):
            r = (st - 3) // 2
            if r > 0:
                softmax_wcs(it)
            s_phase(it, r)
            sqk_phase(it, r)
        elif st in (4, 6):
            r = (st - 4) // 2
            v_phase(it, r)
            vt_phase(it, r)
            agree_phase(it, r)
        elif st == 8:
            v_phase(it, R - 1)
            emit_out(it)

    NST = 9
    for t in range(bl + NST - 1):
        # older items' later (PE-sparse) stages first, then the newest
        # item's dense conv work to keep the PE activity monitor warm
        for st in range(NST - 1, -1, -1):
            it = t - st
            if 0 <= it < bl:
                stage(it, st)
    es.close()


def _pack_consts(inputs):
    conv1_w = np.ascontiguousarray(np.asarray(inputs["conv1_w"], np.float32))
    conv1_b = np.asarray(inputs["conv1_b"], np.float32)
    prim_w = np.ascontiguousarray(np.asarray(inputs["prim_w"], np.float32))
    prim_b = np.asarray(inputs["prim_b"], np.float32)
    W = np.asarray(inputs["W"], np.float32)

    w1 = np.zeros((128, 1152), np.float32)
    for t in range(9):
        for ec in range(4):
            w1[:, t * 128 + ec * 32 : t * 128 + (ec + 1) * 32] = conv1_w[
                :, ec * 128 : (ec + 1) * 128, t
            ].T
    w2 = np.zeros((32, 2304), np.float32)
    for t in range(9):
        w2[:, t * 256 : (t + 1) * 256] = prim_w[:, :, t].T
    wfr = W[0].transpose(0, 2, 1, 3).reshape(U, C, K * C)  # [u, c', (k c)]
    wf = np.zeros((128, 576), np.float32)
    for ch in range(2):
        wf[:, ch * 288 : (ch + 1) * 288] = wfr[ch * 4 : (ch + 1) * 4].reshape(128, 288)
    w9 = wf / 9.0
    b1 = conv1_b.reshape(32, 1).copy()
    b2 = prim_b.reshape(2, 128).T.copy()
    ident = np.eye(128, dtype=np.float32)

    ind_sq8 = np.zeros((128, 16), np.float32)
    for p in range(128):
        ind_sq8[p, p // 32] = 1.0
        ind_sq8[p, 12 + p // 32] = 1.0
    indT8 = np.zeros((8, 256), np.float32)
    for p in range(128):
        indT8[p // 32, p] = 1.0
        indT8[4 + p // 32, 128 + p] = 1.0
    kind = np.zeros((128, 36), np.float32)
    for kg in range(3):
        m = 128 if kg < 2 else 32
        for p in range(m):
            kind[p, kg * 12 + kg * 4 + p // 32] = 1.0
    kindT = np.zeros((12, 288), np.float32)
    for kg in range(3):
        m = 128 if kg < 2 else 32
        for p in range(m):
            kindT[kg * 4 + p // 32, kg * 128 + p] = 1.0
    uind = np.zeros((128, 4), np.float32)
    for p in range(128):
        uind[p, p // 32] = 1.0
    u4T = np.zeros((4, 128), np.float32)
    for p in range(128):
        u4T[p // 32, p] = 1.0

    ind36 = np.zeros((36, 256), np.float32)
    for ch in range(2):
        for p in range(128):
            ind36[ch * 32 + p // 32, ch * 128 + p] = 1.0

    fbc = np.zeros((128, 2), np.float32)
    fbc[:, 0] = 1e-8
    fbc[:, 1] = 1.0

    return {
        "w1": w1, "w2": w2, "wf": wf, "w9": w9, "b1": b1, "b2": b2,
        "ident": ident, "ind_sq8": ind_sq8, "indT8": indT8, "kind": kind,
        "kindT": kindT, "uind": uind, "u4T": u4T, "fb": fbc, "ind36": ind36,
    }


_NC_CACHE = {}


def build_nc(bl=BL):
    if bl in _NC_CACHE:
        return _NC_CACHE[bl]
    nc = bacc.Bacc(
        "TRN2", target_bir_lowering=False, debug=False, num_devices=NCORES
    )
    shapes = {
        "x": ([bl, L], I32), "mask": ([bl, L], F32), "emb": ([V, E], F32),
        "w1": ([128, 1152], F32R), "w2": ([32, 2304], F32R), "wf": ([128, 576], F32R),
        "w9": ([128, 576], F32R), "b1": ([32, 1], F32), "b2": ([128, 2], F32),
        "ident": ([128, 128], F32R), "ind_sq8": ([128, 16], F32R),
        "indT8": ([8, 256], F32R), "kind": ([128, 36], F32R),
        "kindT": ([12, 288], F32R), "uind": ([128, 4], F32R), "u4T": ([4, 128], F32R),
        "fb": ([128, 2], F32), "ind36": ([36, 256], F32R),
    }
    aps = {
        name: nc.dram_tensor(name, shp, dt, kind="ExternalInput").ap()
        for name, (shp, dt) in shapes.items()
    }
    aps["out"] = nc.dram_tensor("out", [bl, K * C], F32, kind="ExternalOutput").ap()
    with tile.TileContext(nc) as tc:
        _emit(tc, nc, aps, bl)
    nc.compile()
    _NC_CACHE[bl] = nc
    return nc


def make_in_maps(inputs, bl=BL, ncores=NCORES):
    consts = _pack_consts(inputs)
    x = np.ascontiguousarray(np.asarray(inputs["x"], np.int32).reshape(ncores, bl, L))
    mask = np.ascontiguousarray(
        np.asarray(inputs["attention_mask"], np.float32).reshape(ncores, bl, L)
    )
    emb = np.ascontiguousarray(np.asarray(inputs["emb"], np.float32))
    return [
        {"x": x[i], "mask": mask[i], "emb": emb, **consts} for i in range(ncores)
    ]


def kernel(x, attention_mask, emb, conv1_w, conv1_b, prim_w, prim_b, W):
    inputs = {
        "x": x, "attention_mask": attention_mask, "emb": emb,
        "conv1_w": conv1_w, "conv1_b": conv1_b,
        "prim_w": prim_w, "prim_b": prim_b, "W": W,
    }
    nc = build_nc(BL)
    in_maps = make_in_maps(inputs)
    res = run_bass_kernel_spmd(nc, in_maps, core_ids=list(range(NCORES)))
    out = np.concatenate(
        [res.results[i]["out"].reshape(BL, K, C) for i in range(NCORES)], axis=0
    )
    return out.astype(np.float32)
